# revision 37
# baseline (speedup 1.0000x reference)
# CapsuleLayer dynamic-routing kernel for 8x Trainium2 NeuronCores — v2.
#
# Problem: u_hat[b,n,m,d] = sum_i W[n,m,d,i] * x[b,m,i]; 3 routing iterations
#   c = softmax_n(blog); s[b,n,d] = sum_m c*u_hat; out = squash_d(s);
#   blog += sum_d out*u_hat
# with B=128, M=2048, I=8, N=32, D=16.
#
# Sharding: M (input capsules) split across 8 cores, 256 per core; only the
# small s[b,n,d] partial sums cross cores (AllReduce) once per iteration.
#
# v2 layout: m on SBUF partitions (two chunks of 128), fp16 compute tensors.
# Per routing pass k>0:
#   phi_{n,i}[m,b] = sum_d W[n,m,d,i]*Rsum[b,n,d]      (PE, K=32 masked-pair)
#   tmp  = phi (PSUM->SBUF f16 copy on Act)
#   tmp2 = tmp * xT                                     (DVE, fp16 2x mode)
#   blogT_n[m,b] = sum_i tmp2                           (DVE tree-add, 2x)
#   expT = exp(blogT)  [Act];  Z = sum_n expT  [DVE tree];  xr = xT / Z
#   z_n[m,(i,b)] = expT_n * xr                          (DVE 2x)
#   sT[(j,d),b] += w0[m,(i,n,d)]^T @ z_n                (PE fp16, PSUM acc)
# s AllReduce in [(j,d),(q,b)] layout (n = q*8+j), squash in-place, Rsum^T
# rebuilt via a DRAM round trip into the zero-masked rtA/rtB pair tiles.

import numpy as np

import concourse.bacc as bacc
import concourse.mybir as mybir
import concourse.tile as tile
from concourse.bass_utils import run_bass_kernel_spmd

B = 128          # batch (== SBUF partitions)
MTOT = 2048      # input capsules
I = 8            # input capsule dim
N = 32           # output capsules
D = 16           # output capsule dim
CORES = 8
MC = MTOT // CORES   # 256 input capsules per core
CH = 2               # m chunks of 128 per core
MCH = MC // CH       # 128
ND = N * D           # 512
EPS = 1e-7
ROUTINGS = 3

F32 = mybir.dt.float32
F16 = mybir.dt.float16
ADD = mybir.AluOpType.add
MULT = mybir.AluOpType.mult
AX_X = mybir.AxisListType.X
ACT = mybir.ActivationFunctionType

_CACHE = {}


def _build_nc(debug_outputs: bool = False, no_collective: bool = False, stage: int = 3):
    nc = bacc.Bacc("TRN2", target_bir_lowering=False, debug=False,
                   num_devices=1 if no_collective else CORES)

    xt_d = nc.dram_tensor("xt", [MCH, CH * I * B], F16, kind="ExternalInput").ap()
    xt2_d = nc.dram_tensor("xt2", [MCH, CH * 2048], F16, kind="ExternalInput").ap()
    wphi_d = nc.dram_tensor("wphi", [128, N * CH * MCH], F16, kind="ExternalInput").ap()
    w0_d = nc.dram_tensor("w0", [MCH, CH * I * N * D], F16, kind="ExternalInput").ap()
    bones_d = nc.dram_tensor("bones", [128, 128], F16, kind="ExternalInput").ap()
    out_d = nc.dram_tensor("out_f", [128, ND], F32, kind="ExternalOutput").ap()

    with tile.TileContext(nc) as tc:
        with tc.tile_pool(name="const", bufs=1) as cp, \
             tc.tile_pool(name="work", bufs=1) as wp, \
             tc.tile_pool(name="tmp4", bufs=4) as tp, \
             tc.tile_pool(name="zp", bufs=3) as zp, \
             tc.tile_pool(name="phip", bufs=2, space="PSUM") as pp, \
             tc.tile_pool(name="dram", bufs=2, space="DRAM") as dp:

            # ---- persistent SBUF ----
            xts = [cp.tile([MCH, I * B], F16, tag=f"xt{c}", name=f"xt{c}")
                   for c in range(CH)]
            xt2s = [cp.tile([MCH, 2048], F16, tag=f"xt2{c}", name=f"xt2{c}")
                    for c in range(CH)]
            wphi = cp.tile([128, N * CH * MCH], F16, tag="wphi")
            w0s = [cp.tile([MCH, I * N * D], F16, tag=f"w0{c}", name=f"w0{c}")
                   for c in range(CH)]
            bones = cp.tile([128, 128], F16, tag="bones")
            eps_t = cp.tile([128, 1], F32, tag="eps")
            shf_t = cp.tile([128, 1], F32, tag="shf")
            rtA = wp.tile([128, N * B], F16, tag="rtA")
            rtB = wp.tile([128, N * B], F16, tag="rtB")
            osum = wp.tile([128, ND], F16, tag="osum")
            blogT = [wp.tile([MCH, N * B], F16, tag=f"blogT{c}", name=f"blogT{c}")
                     for c in range(CH)]
            expT = [wp.tile([MCH, N * B], F16, tag=f"expT{c}", name=f"expT{c}")
                    for c in range(CH)]
            xrs = [wp.tile([MCH, I * B], F16, tag=f"xr{c}", name=f"xr{c}")
                   for c in range(CH)]

            for c in range(CH):
                nc.sync.dma_start(xts[c][:], xt_d[:, c * (I * B):(c + 1) * (I * B)])
                nc.sync.dma_start(w0s[c][:], w0_d[:, c * (I * N * D):(c + 1) * (I * N * D)])
            nc.sync.dma_start(bones[:], bones_d)
            for c in range(CH):
                nc.sync.dma_start(xt2s[c][:], xt2_d[:, c * 2048:(c + 1) * 2048])
            nc.sync.dma_start(wphi[:], wphi_d)
            nc.gpsimd.memset(eps_t[:], EPS)
            nc.gpsimd.memset(shf_t[:], -4.0)
            nc.gpsimd.memset(rtA[:].bitcast(F32), 0.0)
            nc.gpsimd.memset(rtB[:].bitcast(F32), 0.0)

            if stage == 1:
                ks = [0]
            elif stage in (15, 2):
                ks = [0, 1]
            else:
                ks = [0, 1, 2]
            last_full = 0 if stage in (1, 15) else ks[-1]
            for k in ks:
                # ---------- logits -> expT (k > 0) ----------
                if k > 0:
                    for c in range(CH):
                        for np_ in range(N // 2):
                            n0 = np_ * 2
                            # phi pair: [m, (q, n2, par, b)]; bank q holds only
                            # tile_position q (both n's of the pair).
                            phi = pp.tile([MCH, 2048], F32, tag="phi", name="phi")
                            for n2 in range(2):
                                n = n0 + n2
                                for i in range(I):
                                    q, par = i // 2, i % 2
                                    rt_src = rtA if par == 0 else rtB
                                    off = q * 512 + n2 * 256 + par * B
                                    nc.tensor.matmul(
                                        phi[:, off:off + B],
                                        lhsT=wphi[32 * q:32 * q + 32,
                                                  (n * CH + c) * MCH:(n * CH + c + 1) * MCH],
                                        rhs=rt_src[32 * q:32 * q + 32, n * B:(n + 1) * B],
                                        start=True, stop=True,
                                        tile_position=(32 * q, 0),
                                    )
                            # contiguous PSUM f32 -> SBUF f16 evacuation (2 n's)
                            tmp = tp.tile([MCH, 2048], F16, tag="tmp4", name="tmp")
                            nc.scalar.copy(tmp[:], phi[:])
                            # tmp2 = tmp * x  (x replicated over n2 host-side)
                            tmp2 = tp.tile([MCH, 2048], F16, tag="tmp2", name="tmp2")
                            nc.vector.tensor_tensor(tmp2[:], tmp[:], xt2s[c][:], MULT)
                            # tree-reduce over i = (q, par): q-halves twice, then par
                            v = tmp2.rearrange("p (q r) -> p q r", q=4)
                            t1 = tp.tile([MCH, 1024], F16, tag="t1", name="t1")
                            nc.vector.tensor_tensor(
                                t1.rearrange("p (q r) -> p q r", q=2),
                                v[:, 0:2], v[:, 2:4], ADD)
                            v = t1.rearrange("p (q r) -> p q r", q=2)
                            t2 = tp.tile([MCH, 512], F16, tag="t2", name="t2")
                            nc.vector.tensor_tensor(
                                t2.unsqueeze(1), v[:, 0:1], v[:, 1:2], ADD)
                            v = t2.rearrange("p (n2 par b) -> p n2 par b", n2=2, par=2)
                            nc.vector.tensor_tensor(
                                blogT[c][:, n0 * B:(n0 + 2) * B]
                                    .rearrange("p (n2 b) -> p n2 b", n2=2).unsqueeze(2),
                                v[:, :, 0:1], v[:, :, 1:2], ADD)
                        nc.scalar.activation(expT[c][:], blogT[c][:], ACT.Exp, bias=shf_t[:])
                        # Z[m, b] = sum_n expT (tree over n-blocks, on GPSIMD)
                        d1 = wp.tile([MCH, 16 * B], F16, tag="d1", name="d1", bufs=1)
                        nc.vector.tensor_tensor(
                            d1[:], expT[c][:, 0:16 * B], expT[c][:, 16 * B:32 * B], ADD)
                        d2 = wp.tile([MCH, 8 * B], F16, tag="d2", name="d2", bufs=1)
                        nc.vector.tensor_tensor(
                            d2[:], d1[:, 0:8 * B], d1[:, 8 * B:16 * B], ADD)
                        d3 = wp.tile([MCH, 4 * B], F16, tag="d3", name="d3", bufs=1)
                        nc.vector.tensor_tensor(
                            d3[:], d2[:, 0:4 * B], d2[:, 4 * B:8 * B], ADD)
                        d4 = wp.tile([MCH, 2 * B], F16, tag="d4", name="d4", bufs=1)
                        nc.vector.tensor_tensor(
                            d4[:], d3[:, 0:2 * B], d3[:, 2 * B:4 * B], ADD)
                        zden = wp.tile([MCH, B], F32, tag="zden", name="zden", bufs=2)
                        nc.vector.tensor_tensor(
                            zden[:], d4[:, 0:B], d4[:, B:2 * B], ADD)
                        rden = wp.tile([MCH, B], F16, tag="rden", name="rden", bufs=2)
                        with nc.allow_low_precision(reason="routing weights tolerate f16"):
                            nc.vector.reciprocal(rden[:], zden[:])
                        nc.vector.tensor_tensor(
                            xrs[c].rearrange("p (i b) -> p i b", i=I),
                            xts[c].rearrange("p (i b) -> p i b", i=I),
                            rden.unsqueeze(1).broadcast_to([MCH, I, B]),
                            MULT,
                        )

                if stage == 15 and k == 1:
                    continue
                # ---------- s^T accumulation ----------
                sT_sb = wp.tile([128, ND], F32, tag="sTsb", bufs=1)
                if k == 0:
                    # uniform c: rhs (xt) is n-independent -> pack 8 n's in
                    # the stationary: lhsT [m, (n8, d)] -> out [(j,d), b]
                    for g in range(N // 8):
                        sacc8_t = pp.tile([MCH, 2048], F32, tag="phi", name="sacc8")
                        sacc8 = sacc8_t
                        for c in range(CH):
                            for i in range(I):
                                nc.tensor.matmul(
                                    sacc8[0:128, 0:B],
                                    lhsT=w0s[c][:, i * (N * D) + g * 8 * D:
                                                i * (N * D) + (g + 1) * 8 * D],
                                    rhs=xts[c][:, i * B:(i + 1) * B],
                                    start=(c == 0 and i == 0),
                                    stop=(c == CH - 1 and i == I - 1),
                                )
                        nc.scalar.copy(sT_sb[:, g * B:(g + 1) * B], sacc8[0:128, 0:B])
                else:
                    sT16 = wp.tile([16, N * B], F32, tag="sT16", bufs=1)
                    for np_ in range(N // 2):
                        n0 = np_ * 2
                        sacc_t = pp.tile([MCH, 2048], F32, tag="phi", name="sacc")
                        for n2 in range(2):
                            n = n0 + n2
                            for c in range(CH):
                                zn = zp.tile([MCH, I * B], F16, tag="zn", name="zn")
                                nc.vector.tensor_tensor(
                                    zn.rearrange("p (i b) -> p i b", i=I),
                                    xrs[c].rearrange("p (i b) -> p i b", i=I),
                                    expT[c][:, n * B:(n + 1) * B]
                                        .unsqueeze(1).broadcast_to([MCH, I, B]),
                                    MULT,
                                )
                                for i in range(I):
                                    nc.tensor.matmul(
                                        sacc_t[0:D, n2 * B:(n2 + 1) * B],
                                        lhsT=w0s[c][:, i * (N * D) + n * D:
                                                    i * (N * D) + (n + 1) * D],
                                        rhs=zn[:, i * B:(i + 1) * B],
                                        start=(c == 0 and i == 0),
                                        stop=(c == CH - 1 and i == I - 1),
                                    )
                        nc.scalar.copy(sT16[0:16, n0 * B:(n0 + 2) * B],
                                       sacc_t[0:D, 0:2 * B])
                # ---------- AllReduce of s^T partials ----------
                # ssq ends up [(j, d), (q, b)] with n = q*8 + j for every k.
                ssq = wp.tile([128, ND], F32, tag="ssq", bufs=1)
                if k == 0:
                    s_in = dp.tile([128, ND], F32, tag="sin", bufs=2)
                    s_out = dp.tile([128, ND], F32, tag="sout", bufs=2)
                    nc.sync.dma_start(s_in[:], sT_sb[:])
                    if no_collective:
                        nc.sync.dma_start(s_out[:], s_in[:])
                    else:
                        nc.gpsimd.collective_compute(
                            "AllReduce", ADD,
                            replica_groups=[list(range(CORES))],
                            ins=[s_in.opt()],
                            outs=[s_out.opt()],
                        )
                    nc.sync.dma_start(ssq[:], s_out[:])
                else:
                    s_in1 = dp.tile([16, N * B], F32, tag="sin1", bufs=2)
                    s_out1 = dp.tile([16, N * B], F32, tag="sout1", bufs=2)
                    nc.sync.dma_start(s_in1[:], sT16[:])
                    if no_collective:
                        nc.sync.dma_start(s_out1[:], s_in1[:])
                    else:
                        nc.gpsimd.collective_compute(
                            "AllReduce", ADD,
                            replica_groups=[list(range(CORES))],
                            ins=[s_in1.opt()],
                            outs=[s_out1.opt()],
                        )
                    # scatter [d, (q, j, b)] -> [(j, d), (q, b)] during readback
                    nc.sync.dma_start(
                        ssq[:],
                        s_out1.rearrange("d (q j b) -> j d q b", q=4, j=8),
                    )

                # ---------- squash (layout [(j,d), (q,b)], n = q*8+j) ----------
                kscale = (1.0 / N) if k == 0 else 1.0
                sq = wp.tile([128, ND], F16, tag="sqf", bufs=2)
                nc.scalar.activation(sq[:], ssq[:], ACT.Square, scale=kscale)
                s2_t = pp.tile([MCH, 2048], F32, tag="phi", name="ps_sq")
                s2 = s2_t[:, 0:ND]
                nc.tensor.matmul(s2, lhsT=bones[:], rhs=sq[:], start=True, stop=True)
                qq = wp.tile([128, ND], F32, tag="sqz", bufs=2)
                nc.scalar.activation(qq[:], s2, ACT.Sqrt, bias=eps_t[:])
                rr = wp.tile([128, ND], F32, tag="sqz", bufs=2)
                nc.vector.scalar_tensor_tensor(rr[:], s2, 1.0, qq[:], ADD, MULT)
                ww = wp.tile([128, ND], F32, tag="sqz", bufs=2)
                nc.vector.reciprocal(ww[:], rr[:])
                sc = wp.tile([128, ND], F32, tag="sqz", bufs=2)
                nc.vector.tensor_tensor(sc[:], s2, ww[:], MULT)
                o = wp.tile([128, ND], F16 if k < last_full else F32,
                            tag="ot" if k < last_full else "ot32", bufs=2)
                with nc.allow_low_precision(reason="outputs tolerate f16"):
                    nc.vector.scalar_tensor_tensor(o[:], ssq[:], kscale, sc[:], MULT, MULT)

                if k == last_full:
                    nc.sync.dma_start(out_d, o[:])
                if k < ks[-1]:
                    if k == 0:
                        nc.vector.tensor_copy(osum[:], o[:])
                    else:
                        nc.vector.tensor_tensor(osum[:], osum[:], o[:], ADD)
                    # Rsum^T rebuild: osum [(j,d),(q,b)] f16 -> DRAM in
                    # [d, (n,b)] layout (scatter on store), then contiguous
                    # replicating loads into the rt pair tiles.
                    o_dram = dp.tile([D, N * B], F16, tag="osd", bufs=2)
                    nc.sync.dma_start(
                        o_dram.rearrange("d (q j b) -> j d q b", q=4, j=8),
                        osum[:])
                    # rt rows 32q'+16par+d hold RsumT[d, (n,b)]
                    for qq_ in range(4):
                        for par in range(2):
                            rt_dst = rtA if par == 0 else rtB
                            row0 = 32 * qq_ + 16 * par
                            eng = nc.sync if par == 0 else nc.scalar
                            eng.dma_start(rt_dst[row0:row0 + 16, :], o_dram[:, :])

    nc.compile()
    return nc


def _host_prep(inputs: np.ndarray, W: np.ndarray):
    """Build the per-core input maps (all layouts host-side)."""
    inputs = np.ascontiguousarray(inputs, dtype=np.float32)
    W = np.ascontiguousarray(W, dtype=np.float32)
    bones = np.kron(np.eye(8, dtype=np.float32),
                    np.ones((16, 16), dtype=np.float32))
    in_maps = []
    for core in range(CORES):
        xc = inputs[:, core * MC:(core + 1) * MC, :]      # [B, MC, I]
        Wc = W[:, core * MC:(core + 1) * MC, :, :]        # [N, MC, D, I]
        # xt[m, (c, i, b)]: per chunk, (i, b) layout
        xcr = xc.reshape(B, CH, MCH, I)
        xt = xcr.transpose(2, 1, 3, 0).reshape(MCH, CH * I * B)
        # wphi[32q+16par+d, (n, c, m)] = W[n, m, d, i], i = 2q+par
        Wr = Wc.reshape(N, CH, MCH, D, I)
        wphi = np.zeros((4, 2, D, N, CH, MCH), dtype=np.float32)
        for i in range(I):
            q, par = i // 2, i % 2
            wphi[q, par] = Wr[:, :, :, :, i].transpose(3, 0, 1, 2)
        wphi = wphi.reshape(128, N * CH * MCH)
        # xt2[m, (c, q, n2, par, b)]: x replicated over the n-pair dim
        arr = xcr.transpose(2, 1, 3, 0).reshape(MCH, CH, 4, 2, B)
        xt2 = np.broadcast_to(arr[:, :, :, None, :, :],
                              (MCH, CH, 4, 2, 2, B)).reshape(MCH, CH * 2048)
        # w0[m, (c, i, n, d)]
        w0 = Wr.transpose(2, 1, 4, 0, 3).reshape(MCH, CH * I * N * D)
        in_maps.append({
            "xt": np.ascontiguousarray(xt, dtype=np.float16),
            "xt2": np.ascontiguousarray(xt2, dtype=np.float16),
            "wphi": np.ascontiguousarray(wphi, dtype=np.float16),
            "w0": np.ascontiguousarray(w0, dtype=np.float16),
            "bones": bones.astype(np.float16),
        })
    return in_maps


def _decode_out(out_f: np.ndarray) -> np.ndarray:
    # out_f [128, 512] in [(j, d), (q, b)] layout, n = q*8+j -> [b, n, d]
    arr = out_f.astype(np.float32).reshape(8, D, 4, B)    # j, d, q, b
    return np.ascontiguousarray(
        arr.transpose(3, 2, 0, 1).reshape(B, N, D))


def run(inputs: np.ndarray, W: np.ndarray, trace: bool = False):
    key = "nc"
    if key not in _CACHE:
        _CACHE[key] = _build_nc(False)
    nc = _CACHE[key]
    in_maps = _host_prep(inputs, W)
    res = run_bass_kernel_spmd(nc, in_maps, core_ids=list(range(CORES)), trace=trace)
    out = _decode_out(res.results[0]["out_f"])
    return out, res


def kernel(inputs: np.ndarray, W: np.ndarray) -> np.ndarray:
    out, _ = run(inputs, W, trace=False)
    return out


# revision 39
# speedup vs baseline: 1.0147x; 1.0147x over previous
# CapsuleLayer dynamic-routing kernel for 8x Trainium2 NeuronCores — v2.
#
# Problem: u_hat[b,n,m,d] = sum_i W[n,m,d,i] * x[b,m,i]; 3 routing iterations
#   c = softmax_n(blog); s[b,n,d] = sum_m c*u_hat; out = squash_d(s);
#   blog += sum_d out*u_hat
# with B=128, M=2048, I=8, N=32, D=16.
#
# Sharding: M (input capsules) split across 8 cores, 256 per core; only the
# small s[b,n,d] partial sums cross cores (AllReduce) once per iteration.
#
# v2 layout: m on SBUF partitions (two chunks of 128), fp16 compute tensors.
# Per routing pass k>0:
#   phi_{n,i}[m,b] = sum_d W[n,m,d,i]*Rsum[b,n,d]      (PE, K=32 masked-pair)
#   tmp  = phi (PSUM->SBUF f16 copy on Act)
#   tmp2 = tmp * xT                                     (DVE, fp16 2x mode)
#   blogT_n[m,b] = sum_i tmp2                           (DVE tree-add, 2x)
#   expT = exp(blogT)  [Act];  Z = sum_n expT  [DVE tree];  xr = xT / Z
#   z_n[m,(i,b)] = expT_n * xr                          (DVE 2x)
#   sT[(j,d),b] += w0[m,(i,n,d)]^T @ z_n                (PE fp16, PSUM acc)
# s AllReduce in [(j,d),(q,b)] layout (n = q*8+j), squash in-place, Rsum^T
# rebuilt via a DRAM round trip into the zero-masked rtA/rtB pair tiles.

import numpy as np

import concourse.bacc as bacc
import concourse.mybir as mybir
import concourse.tile as tile
from concourse.bass_utils import run_bass_kernel_spmd

B = 128          # batch (== SBUF partitions)
MTOT = 2048      # input capsules
I = 8            # input capsule dim
N = 32           # output capsules
D = 16           # output capsule dim
CORES = 8
MC = MTOT // CORES   # 256 input capsules per core
CH = 2               # m chunks of 128 per core
MCH = MC // CH       # 128
ND = N * D           # 512
EPS = 1e-7
ROUTINGS = 3

F32 = mybir.dt.float32
F16 = mybir.dt.float16
ADD = mybir.AluOpType.add
MULT = mybir.AluOpType.mult
AX_X = mybir.AxisListType.X
ACT = mybir.ActivationFunctionType

_CACHE = {}


def _build_nc(debug_outputs: bool = False, no_collective: bool = False, stage: int = 3):
    nc = bacc.Bacc("TRN2", target_bir_lowering=False, debug=False,
                   num_devices=1 if no_collective else CORES)

    xt_d = nc.dram_tensor("xt", [MCH, CH * I * B], F16, kind="ExternalInput").ap()
    xt2_d = nc.dram_tensor("xt2", [MCH, CH * 2048], F16, kind="ExternalInput").ap()
    wphi_d = nc.dram_tensor("wphi", [128, N * CH * MCH], F16, kind="ExternalInput").ap()
    w0_d = nc.dram_tensor("w0", [MCH, CH * I * N * D], F16, kind="ExternalInput").ap()
    bones_d = nc.dram_tensor("bones", [128, 128], F16, kind="ExternalInput").ap()
    out_d = nc.dram_tensor("out_f", [128, ND], F32, kind="ExternalOutput").ap()

    with tile.TileContext(nc) as tc:
        with tc.tile_pool(name="const", bufs=1) as cp, \
             tc.tile_pool(name="work", bufs=1) as wp, \
             tc.tile_pool(name="tmp4", bufs=4) as tp, \
             tc.tile_pool(name="zp", bufs=3) as zp, \
             tc.tile_pool(name="phip", bufs=2, space="PSUM") as pp, \
             tc.tile_pool(name="dram", bufs=2, space="DRAM") as dp:

            # ---- persistent SBUF ----
            xts = [cp.tile([MCH, I * B], F16, tag=f"xt{c}", name=f"xt{c}")
                   for c in range(CH)]
            xt2s = [cp.tile([MCH, 2048], F16, tag=f"xt2{c}", name=f"xt2{c}")
                    for c in range(CH)]
            wphi = cp.tile([128, N * CH * MCH], F16, tag="wphi")
            w0s = [cp.tile([MCH, I * N * D], F16, tag=f"w0{c}", name=f"w0{c}")
                   for c in range(CH)]
            bones = cp.tile([128, 128], F16, tag="bones")
            eps_t = cp.tile([128, 1], F32, tag="eps")
            shf_t = cp.tile([128, 1], F32, tag="shf")
            rtA = wp.tile([128, N * B], F16, tag="rtA")
            rtB = wp.tile([128, N * B], F16, tag="rtB")
            osum = wp.tile([128, ND], F16, tag="osum")
            blogT = [wp.tile([MCH, N * B], F16, tag=f"blogT{c}", name=f"blogT{c}")
                     for c in range(CH)]
            expT = [wp.tile([MCH, N * B], F16, tag=f"expT{c}", name=f"expT{c}")
                    for c in range(CH)]
            xrs = [wp.tile([MCH, I * B], F16, tag=f"xr{c}", name=f"xr{c}")
                   for c in range(CH)]

            for c in range(CH):
                nc.sync.dma_start(xts[c][:], xt_d[:, c * (I * B):(c + 1) * (I * B)])
                nc.sync.dma_start(w0s[c][:], w0_d[:, c * (I * N * D):(c + 1) * (I * N * D)])
            nc.sync.dma_start(bones[:], bones_d)
            for c in range(CH):
                nc.sync.dma_start(xt2s[c][:], xt2_d[:, c * 2048:(c + 1) * 2048])
            nc.sync.dma_start(wphi[:], wphi_d)
            nc.gpsimd.memset(eps_t[:], EPS)
            nc.gpsimd.memset(shf_t[:], -4.0)
            nc.gpsimd.memset(rtA[:].bitcast(F32), 0.0)
            nc.gpsimd.memset(rtB[:].bitcast(F32), 0.0)

            if stage == 1:
                ks = [0]
            elif stage in (15, 2):
                ks = [0, 1]
            else:
                ks = [0, 1, 2]
            last_full = 0 if stage in (1, 15) else ks[-1]
            for k in ks:
                # ---------- logits -> expT (k > 0) ----------
                if k > 0:
                    for c in range(CH):
                        for np_ in range(N // 2):
                            n0 = np_ * 2
                            # phi pair: [m, (q, n2, par, b)]; bank q holds only
                            # tile_position q (both n's of the pair).
                            phi = pp.tile([MCH, 2048], F32, tag="phi", name="phi")
                            for n2 in range(2):
                                n = n0 + n2
                                for i in range(I):
                                    q, par = i // 2, i % 2
                                    rt_src = rtA if par == 0 else rtB
                                    off = q * 512 + n2 * 256 + par * B
                                    nc.tensor.matmul(
                                        phi[:, off:off + B],
                                        lhsT=wphi[32 * q:32 * q + 32,
                                                  (n * CH + c) * MCH:(n * CH + c + 1) * MCH],
                                        rhs=rt_src[32 * q:32 * q + 32, n * B:(n + 1) * B],
                                        start=True, stop=True,
                                        tile_position=(32 * q, 0),
                                    )
                            # contiguous PSUM f32 -> SBUF f16 evacuation (2 n's)
                            tmp = tp.tile([MCH, 2048], F16, tag="tmp4", name="tmp")
                            nc.scalar.copy(tmp[:], phi[:])
                            # tmp2 = tmp * x  (x replicated over n2 host-side)
                            tmp2 = tp.tile([MCH, 2048], F16, tag="tmp2", name="tmp2")
                            nc.vector.tensor_tensor(tmp2[:], tmp[:], xt2s[c][:], MULT)
                            # tree-reduce over i = (q, par): q-halves twice, then par
                            v = tmp2.rearrange("p (q r) -> p q r", q=4)
                            t1 = tp.tile([MCH, 1024], F16, tag="t1", name="t1")
                            nc.vector.tensor_tensor(
                                t1.rearrange("p (q r) -> p q r", q=2),
                                v[:, 0:2], v[:, 2:4], ADD)
                            v = t1.rearrange("p (q r) -> p q r", q=2)
                            t2 = tp.tile([MCH, 512], F16, tag="t2", name="t2")
                            nc.vector.tensor_tensor(
                                t2.unsqueeze(1), v[:, 0:1], v[:, 1:2], ADD)
                            v = t2.rearrange("p (n2 par b) -> p n2 par b", n2=2, par=2)
                            nc.vector.tensor_tensor(
                                blogT[c][:, n0 * B:(n0 + 2) * B]
                                    .rearrange("p (n2 b) -> p n2 b", n2=2).unsqueeze(2),
                                v[:, :, 0:1], v[:, :, 1:2], ADD)
                            if np_ % 4 == 3:
                                qx = np_ // 4
                                nc.scalar.activation(
                                    expT[c][:, qx * 8 * B:(qx + 1) * 8 * B],
                                    blogT[c][:, qx * 8 * B:(qx + 1) * 8 * B],
                                    ACT.Exp, bias=shf_t[:])
                                if qx == 1:
                                    dta = wp.tile([MCH, 8 * B], F16, tag="dta",
                                                  name="dta", bufs=2)
                                    nc.vector.tensor_tensor(
                                        dta[:], expT[c][:, 0:8 * B],
                                        expT[c][:, 8 * B:16 * B], ADD)
                                elif qx == 3:
                                    dtb = wp.tile([MCH, 8 * B], F16, tag="dtb",
                                                  name="dtb", bufs=2)
                                    nc.vector.tensor_tensor(
                                        dtb[:], expT[c][:, 16 * B:24 * B],
                                        expT[c][:, 24 * B:32 * B], ADD)
                        d2 = wp.tile([MCH, 8 * B], F16, tag="d2", name="d2", bufs=2)
                        nc.vector.tensor_tensor(d2[:], dta[:], dtb[:], ADD)
                        d3 = wp.tile([MCH, 4 * B], F16, tag="d3", name="d3", bufs=1)
                        nc.vector.tensor_tensor(
                            d3[:], d2[:, 0:4 * B], d2[:, 4 * B:8 * B], ADD)
                        d4 = wp.tile([MCH, 2 * B], F16, tag="d4", name="d4", bufs=1)
                        nc.vector.tensor_tensor(
                            d4[:], d3[:, 0:2 * B], d3[:, 2 * B:4 * B], ADD)
                        zden = wp.tile([MCH, B], F32, tag="zden", name="zden", bufs=2)
                        nc.vector.tensor_tensor(
                            zden[:], d4[:, 0:B], d4[:, B:2 * B], ADD)
                        rden = wp.tile([MCH, B], F16, tag="rden", name="rden", bufs=2)
                        with nc.allow_low_precision(reason="routing weights tolerate f16"):
                            nc.vector.reciprocal(rden[:], zden[:])
                        nc.vector.tensor_tensor(
                            xrs[c].rearrange("p (i b) -> p i b", i=I),
                            xts[c].rearrange("p (i b) -> p i b", i=I),
                            rden.unsqueeze(1).broadcast_to([MCH, I, B]),
                            MULT,
                        )

                if stage == 15 and k == 1:
                    continue
                # ---------- s^T accumulation ----------
                sT_sb = wp.tile([128, ND], F32, tag="sTsb", bufs=1)
                if k == 0:
                    # uniform c: rhs (xt) is n-independent -> pack 8 n's in
                    # the stationary: lhsT [m, (n8, d)] -> out [(j,d), b]
                    for g in range(N // 8):
                        sacc8_t = pp.tile([MCH, 2048], F32, tag="phi", name="sacc8")
                        sacc8 = sacc8_t
                        for c in range(CH):
                            for i in range(I):
                                nc.tensor.matmul(
                                    sacc8[0:128, 0:B],
                                    lhsT=w0s[c][:, i * (N * D) + g * 8 * D:
                                                i * (N * D) + (g + 1) * 8 * D],
                                    rhs=xts[c][:, i * B:(i + 1) * B],
                                    start=(c == 0 and i == 0),
                                    stop=(c == CH - 1 and i == I - 1),
                                )
                        nc.scalar.copy(sT_sb[:, g * B:(g + 1) * B], sacc8[0:128, 0:B])
                else:
                    sT16 = wp.tile([16, N * B], F32, tag="sT16", bufs=1)
                    for np_ in range(N // 2):
                        n0 = np_ * 2
                        sacc_t = pp.tile([MCH, 2048], F32, tag="phi", name="sacc")
                        for n2 in range(2):
                            n = n0 + n2
                            for c in range(CH):
                                zn = zp.tile([MCH, I * B], F16, tag="zn", name="zn")
                                nc.vector.tensor_tensor(
                                    zn.rearrange("p (i b) -> p i b", i=I),
                                    xrs[c].rearrange("p (i b) -> p i b", i=I),
                                    expT[c][:, n * B:(n + 1) * B]
                                        .unsqueeze(1).broadcast_to([MCH, I, B]),
                                    MULT,
                                )
                                for i in range(I):
                                    nc.tensor.matmul(
                                        sacc_t[0:D, n2 * B:(n2 + 1) * B],
                                        lhsT=w0s[c][:, i * (N * D) + n * D:
                                                    i * (N * D) + (n + 1) * D],
                                        rhs=zn[:, i * B:(i + 1) * B],
                                        start=(c == 0 and i == 0),
                                        stop=(c == CH - 1 and i == I - 1),
                                    )
                        nc.scalar.copy(sT16[0:16, n0 * B:(n0 + 2) * B],
                                       sacc_t[0:D, 0:2 * B])
                # ---------- AllReduce of s^T partials ----------
                # ssq ends up [(j, d), (q, b)] with n = q*8 + j for every k.
                ssq = wp.tile([128, ND], F32, tag="ssq", bufs=1)
                if k == 0:
                    s_in = dp.tile([128, ND], F32, tag="sin", bufs=2)
                    s_out = dp.tile([128, ND], F32, tag="sout", bufs=2)
                    nc.sync.dma_start(s_in[:], sT_sb[:])
                    if no_collective:
                        nc.sync.dma_start(s_out[:], s_in[:])
                    else:
                        nc.gpsimd.collective_compute(
                            "AllReduce", ADD,
                            replica_groups=[list(range(CORES))],
                            ins=[s_in.opt()],
                            outs=[s_out.opt()],
                        )
                    nc.sync.dma_start(ssq[:], s_out[:])
                else:
                    s_in1 = dp.tile([16, N * B], F32, tag="sin1", bufs=2)
                    s_out1 = dp.tile([16, N * B], F32, tag="sout1", bufs=2)
                    nc.sync.dma_start(s_in1[:], sT16[:])
                    if no_collective:
                        nc.sync.dma_start(s_out1[:], s_in1[:])
                    else:
                        nc.gpsimd.collective_compute(
                            "AllReduce", ADD,
                            replica_groups=[list(range(CORES))],
                            ins=[s_in1.opt()],
                            outs=[s_out1.opt()],
                        )
                    # scatter [d, (q, j, b)] -> [(j, d), (q, b)] during readback
                    nc.sync.dma_start(
                        ssq[:],
                        s_out1.rearrange("d (q j b) -> j d q b", q=4, j=8),
                    )

                # ---------- squash (layout [(j,d), (q,b)], n = q*8+j) ----------
                kscale = (1.0 / N) if k == 0 else 1.0
                sq = wp.tile([128, ND], F16, tag="sqf", bufs=2)
                nc.scalar.activation(sq[:], ssq[:], ACT.Square, scale=kscale)
                s2_t = pp.tile([MCH, 2048], F32, tag="phi", name="ps_sq")
                s2 = s2_t[:, 0:ND]
                nc.tensor.matmul(s2, lhsT=bones[:], rhs=sq[:], start=True, stop=True)
                qq = wp.tile([128, ND], F32, tag="sqz", bufs=2)
                nc.scalar.activation(qq[:], s2, ACT.Sqrt, bias=eps_t[:])
                rr = wp.tile([128, ND], F32, tag="sqz", bufs=2)
                nc.vector.scalar_tensor_tensor(rr[:], s2, 1.0, qq[:], ADD, MULT)
                ww = wp.tile([128, ND], F32, tag="sqz", bufs=2)
                nc.vector.reciprocal(ww[:], rr[:])
                sc = wp.tile([128, ND], F32, tag="sqz", bufs=2)
                nc.vector.tensor_tensor(sc[:], s2, ww[:], MULT)
                o = wp.tile([128, ND], F16 if k < last_full else F32,
                            tag="ot" if k < last_full else "ot32", bufs=2)
                with nc.allow_low_precision(reason="outputs tolerate f16"):
                    nc.vector.scalar_tensor_tensor(o[:], ssq[:], kscale, sc[:], MULT, MULT)

                if k == last_full:
                    nc.sync.dma_start(out_d, o[:])
                if k < ks[-1]:
                    if k == 0:
                        nc.vector.tensor_copy(osum[:], o[:])
                    else:
                        nc.vector.tensor_tensor(osum[:], osum[:], o[:], ADD)
                    # Rsum^T rebuild: osum [(j,d),(q,b)] f16 -> DRAM in
                    # [d, (n,b)] layout (scatter on store), then contiguous
                    # replicating loads into the rt pair tiles.
                    o_dram = dp.tile([D, N * B], F16, tag="osd", bufs=2)
                    nc.sync.dma_start(
                        o_dram.rearrange("d (q j b) -> j d q b", q=4, j=8),
                        osum[:])
                    # rt rows 32q'+16par+d hold RsumT[d, (n,b)]
                    for qq_ in range(4):
                        for par in range(2):
                            rt_dst = rtA if par == 0 else rtB
                            row0 = 32 * qq_ + 16 * par
                            eng = nc.sync if par == 0 else nc.scalar
                            eng.dma_start(rt_dst[row0:row0 + 16, :], o_dram[:, :])

    nc.compile()
    return nc


def _host_prep(inputs: np.ndarray, W: np.ndarray):
    """Build the per-core input maps (all layouts host-side)."""
    inputs = np.ascontiguousarray(inputs, dtype=np.float32)
    W = np.ascontiguousarray(W, dtype=np.float32)
    bones = np.kron(np.eye(8, dtype=np.float32),
                    np.ones((16, 16), dtype=np.float32))
    in_maps = []
    for core in range(CORES):
        xc = inputs[:, core * MC:(core + 1) * MC, :]      # [B, MC, I]
        Wc = W[:, core * MC:(core + 1) * MC, :, :]        # [N, MC, D, I]
        # xt[m, (c, i, b)]: per chunk, (i, b) layout
        xcr = xc.reshape(B, CH, MCH, I)
        xt = xcr.transpose(2, 1, 3, 0).reshape(MCH, CH * I * B)
        # wphi[32q+16par+d, (n, c, m)] = W[n, m, d, i], i = 2q+par
        Wr = Wc.reshape(N, CH, MCH, D, I)
        wphi = np.zeros((4, 2, D, N, CH, MCH), dtype=np.float32)
        for i in range(I):
            q, par = i // 2, i % 2
            wphi[q, par] = Wr[:, :, :, :, i].transpose(3, 0, 1, 2)
        wphi = wphi.reshape(128, N * CH * MCH)
        # xt2[m, (c, q, n2, par, b)]: x replicated over the n-pair dim
        arr = xcr.transpose(2, 1, 3, 0).reshape(MCH, CH, 4, 2, B)
        xt2 = np.broadcast_to(arr[:, :, :, None, :, :],
                              (MCH, CH, 4, 2, 2, B)).reshape(MCH, CH * 2048)
        # w0[m, (c, i, n, d)]
        w0 = Wr.transpose(2, 1, 4, 0, 3).reshape(MCH, CH * I * N * D)
        in_maps.append({
            "xt": np.ascontiguousarray(xt, dtype=np.float16),
            "xt2": np.ascontiguousarray(xt2, dtype=np.float16),
            "wphi": np.ascontiguousarray(wphi, dtype=np.float16),
            "w0": np.ascontiguousarray(w0, dtype=np.float16),
            "bones": bones.astype(np.float16),
        })
    return in_maps


def _decode_out(out_f: np.ndarray) -> np.ndarray:
    # out_f [128, 512] in [(j, d), (q, b)] layout, n = q*8+j -> [b, n, d]
    arr = out_f.astype(np.float32).reshape(8, D, 4, B)    # j, d, q, b
    return np.ascontiguousarray(
        arr.transpose(3, 2, 0, 1).reshape(B, N, D))


def run(inputs: np.ndarray, W: np.ndarray, trace: bool = False):
    key = "nc"
    if key not in _CACHE:
        _CACHE[key] = _build_nc(False)
    nc = _CACHE[key]
    in_maps = _host_prep(inputs, W)
    res = run_bass_kernel_spmd(nc, in_maps, core_ids=list(range(CORES)), trace=trace)
    out = _decode_out(res.results[0]["out_f"])
    return out, res


def kernel(inputs: np.ndarray, W: np.ndarray) -> np.ndarray:
    out, _ = run(inputs, W, trace=False)
    return out


# revision 40
# speedup vs baseline: 1.0193x; 1.0046x over previous
# CapsuleLayer dynamic-routing kernel for 8x Trainium2 NeuronCores — v2.
#
# Problem: u_hat[b,n,m,d] = sum_i W[n,m,d,i] * x[b,m,i]; 3 routing iterations
#   c = softmax_n(blog); s[b,n,d] = sum_m c*u_hat; out = squash_d(s);
#   blog += sum_d out*u_hat
# with B=128, M=2048, I=8, N=32, D=16.
#
# Sharding: M (input capsules) split across 8 cores, 256 per core; only the
# small s[b,n,d] partial sums cross cores (AllReduce) once per iteration.
#
# v2 layout: m on SBUF partitions (two chunks of 128), fp16 compute tensors.
# Per routing pass k>0:
#   phi_{n,i}[m,b] = sum_d W[n,m,d,i]*Rsum[b,n,d]      (PE, K=32 masked-pair)
#   tmp  = phi (PSUM->SBUF f16 copy on Act)
#   tmp2 = tmp * xT                                     (DVE, fp16 2x mode)
#   blogT_n[m,b] = sum_i tmp2                           (DVE tree-add, 2x)
#   expT = exp(blogT)  [Act];  Z = sum_n expT  [DVE tree];  xr = xT / Z
#   z_n[m,(i,b)] = expT_n * xr                          (DVE 2x)
#   sT[(j,d),b] += w0[m,(i,n,d)]^T @ z_n                (PE fp16, PSUM acc)
# s AllReduce in [(j,d),(q,b)] layout (n = q*8+j), squash in-place, Rsum^T
# rebuilt via a DRAM round trip into the zero-masked rtA/rtB pair tiles.

import numpy as np

import concourse.bacc as bacc
import concourse.mybir as mybir
import concourse.tile as tile
from concourse.bass_utils import run_bass_kernel_spmd

B = 128          # batch (== SBUF partitions)
MTOT = 2048      # input capsules
I = 8            # input capsule dim
N = 32           # output capsules
D = 16           # output capsule dim
CORES = 8
MC = MTOT // CORES   # 256 input capsules per core
CH = 2               # m chunks of 128 per core
MCH = MC // CH       # 128
ND = N * D           # 512
EPS = 1e-7
ROUTINGS = 3

F32 = mybir.dt.float32
F16 = mybir.dt.float16
ADD = mybir.AluOpType.add
MULT = mybir.AluOpType.mult
AX_X = mybir.AxisListType.X
ACT = mybir.ActivationFunctionType

_CACHE = {}


def _build_nc(debug_outputs: bool = False, no_collective: bool = False, stage: int = 3):
    nc = bacc.Bacc("TRN2", target_bir_lowering=False, debug=False,
                   num_devices=1 if no_collective else CORES)

    xt_d = nc.dram_tensor("xt", [MCH, CH * I * B], F16, kind="ExternalInput").ap()
    xt2_d = nc.dram_tensor("xt2", [MCH, CH * 2048], F16, kind="ExternalInput").ap()
    wphi_d = nc.dram_tensor("wphi", [128, N * CH * MCH], F16, kind="ExternalInput").ap()
    w0_d = nc.dram_tensor("w0", [MCH, CH * I * N * D], F16, kind="ExternalInput").ap()
    bones_d = nc.dram_tensor("bones", [128, 128], F16, kind="ExternalInput").ap()
    out_d = nc.dram_tensor("out_f", [128, ND], F32, kind="ExternalOutput").ap()

    with tile.TileContext(nc) as tc:
        with tc.tile_pool(name="const", bufs=1) as cp, \
             tc.tile_pool(name="work", bufs=1) as wp, \
             tc.tile_pool(name="tmp4", bufs=4) as tp, \
             tc.tile_pool(name="zp", bufs=3) as zp, \
             tc.tile_pool(name="phip", bufs=2, space="PSUM") as pp, \
             tc.tile_pool(name="dram", bufs=2, space="DRAM") as dp:

            # ---- persistent SBUF ----
            xts = [cp.tile([MCH, I * B], F16, tag=f"xt{c}", name=f"xt{c}")
                   for c in range(CH)]
            xt2s = [cp.tile([MCH, 2048], F16, tag=f"xt2{c}", name=f"xt2{c}")
                    for c in range(CH)]
            wphi = cp.tile([128, N * CH * MCH], F16, tag="wphi")
            w0s = [cp.tile([MCH, I * N * D], F16, tag=f"w0{c}", name=f"w0{c}")
                   for c in range(CH)]
            bones = cp.tile([128, 128], F16, tag="bones")
            eps_t = cp.tile([128, 1], F32, tag="eps")
            shf_t = cp.tile([128, 1], F32, tag="shf")
            rtA = wp.tile([128, N * B], F16, tag="rtA")
            rtB = wp.tile([128, N * B], F16, tag="rtB")
            osum = wp.tile([128, ND], F16, tag="osum")
            blogT = [wp.tile([MCH, N * B], F16, tag=f"blogT{c}", name=f"blogT{c}")
                     for c in range(CH)]
            expT = [wp.tile([MCH, N * B], F16, tag=f"expT{c}", name=f"expT{c}")
                    for c in range(CH)]
            xrs = [wp.tile([MCH, I * B], F16, tag=f"xr{c}", name=f"xr{c}")
                   for c in range(CH)]

            for c in range(CH):
                nc.sync.dma_start(xts[c][:], xt_d[:, c * (I * B):(c + 1) * (I * B)])
                nc.sync.dma_start(w0s[c][:], w0_d[:, c * (I * N * D):(c + 1) * (I * N * D)])
            nc.sync.dma_start(bones[:], bones_d)
            for c in range(CH):
                nc.sync.dma_start(xt2s[c][:], xt2_d[:, c * 2048:(c + 1) * 2048])
            nc.sync.dma_start(wphi[:], wphi_d)
            nc.gpsimd.memset(eps_t[:], EPS)
            nc.gpsimd.memset(shf_t[:], -4.0)
            nc.gpsimd.memset(rtA[:].bitcast(F32), 0.0)
            nc.gpsimd.memset(rtB[:].bitcast(F32), 0.0)

            if stage == 1:
                ks = [0]
            elif stage in (15, 2):
                ks = [0, 1]
            else:
                ks = [0, 1, 2]
            last_full = 0 if stage in (1, 15) else ks[-1]
            for k in ks:
                # ---------- logits -> expT (k > 0) ----------
                if k > 0:
                    for c in range(CH):
                        for np_ in range(N // 2):
                            n0 = np_ * 2
                            # phi pair: [m, (q, n2, par, b)]; bank q holds only
                            # tile_position q (both n's of the pair).
                            phi = pp.tile([MCH, 2048], F32, tag="phi", name="phi")
                            for n2 in range(2):
                                n = n0 + n2
                                for i in range(I):
                                    q, par = i // 2, i % 2
                                    rt_src = rtA if par == 0 else rtB
                                    off = q * 512 + n2 * 256 + par * B
                                    nc.tensor.matmul(
                                        phi[:, off:off + B],
                                        lhsT=wphi[32 * q:32 * q + 32,
                                                  (n * CH + c) * MCH:(n * CH + c + 1) * MCH],
                                        rhs=rt_src[32 * q:32 * q + 32, n * B:(n + 1) * B],
                                        start=True, stop=True,
                                        tile_position=(32 * q, 0),
                                    )
                            # contiguous PSUM f32 -> SBUF f16 evacuation (2 n's)
                            tmp = tp.tile([MCH, 2048], F16, tag="tmp4", name="tmp")
                            nc.scalar.copy(tmp[:], phi[:])
                            # tmp2 = tmp * x  (x replicated over n2 host-side)
                            tmp2 = tp.tile([MCH, 2048], F16, tag="tmp2", name="tmp2")
                            nc.vector.tensor_tensor(tmp2[:], tmp[:], xt2s[c][:], MULT)
                            # tree-reduce over i = (q, par): q-halves twice, then par
                            v = tmp2.rearrange("p (q r) -> p q r", q=4)
                            t1 = tp.tile([MCH, 1024], F16, tag="t1", name="t1")
                            nc.vector.tensor_tensor(
                                t1.rearrange("p (q r) -> p q r", q=2),
                                v[:, 0:2], v[:, 2:4], ADD)
                            v = t1.rearrange("p (q r) -> p q r", q=2)
                            t2 = tp.tile([MCH, 512], F16, tag="t2", name="t2")
                            nc.vector.tensor_tensor(
                                t2.unsqueeze(1), v[:, 0:1], v[:, 1:2], ADD)
                            v = t2.rearrange("p (n2 par b) -> p n2 par b", n2=2, par=2)
                            nc.vector.tensor_tensor(
                                blogT[c][:, n0 * B:(n0 + 2) * B]
                                    .rearrange("p (n2 b) -> p n2 b", n2=2).unsqueeze(2),
                                v[:, :, 0:1], v[:, :, 1:2], ADD)
                            if np_ % 4 == 3:
                                qx = np_ // 4
                                nc.scalar.activation(
                                    expT[c][:, qx * 8 * B:(qx + 1) * 8 * B],
                                    blogT[c][:, qx * 8 * B:(qx + 1) * 8 * B],
                                    ACT.Exp, bias=shf_t[:])
                                if qx == 1:
                                    dta = wp.tile([MCH, 8 * B], F16, tag="dta",
                                                  name="dta", bufs=2)
                                    nc.vector.tensor_tensor(
                                        dta[:], expT[c][:, 0:8 * B],
                                        expT[c][:, 8 * B:16 * B], ADD)
                                elif qx == 3:
                                    dtb = wp.tile([MCH, 8 * B], F16, tag="dtb",
                                                  name="dtb", bufs=2)
                                    nc.vector.tensor_tensor(
                                        dtb[:], expT[c][:, 16 * B:24 * B],
                                        expT[c][:, 24 * B:32 * B], ADD)
                        d2 = wp.tile([MCH, 8 * B], F16, tag="d2", name="d2", bufs=2)
                        nc.vector.tensor_tensor(d2[:], dta[:], dtb[:], ADD)
                        d3 = wp.tile([MCH, 4 * B], F16, tag="d3", name="d3", bufs=1)
                        nc.vector.tensor_tensor(
                            d3[:], d2[:, 0:4 * B], d2[:, 4 * B:8 * B], ADD)
                        d4 = wp.tile([MCH, 2 * B], F16, tag="d4", name="d4", bufs=1)
                        nc.vector.tensor_tensor(
                            d4[:], d3[:, 0:2 * B], d3[:, 2 * B:4 * B], ADD)
                        zden = wp.tile([MCH, B], F32, tag="zden", name="zden", bufs=2)
                        nc.vector.tensor_tensor(
                            zden[:], d4[:, 0:B], d4[:, B:2 * B], ADD)
                        rden = wp.tile([MCH, B], F16, tag="rden", name="rden", bufs=2)
                        with nc.allow_low_precision(reason="routing weights tolerate f16"):
                            nc.vector.reciprocal(rden[:], zden[:])
                        nc.vector.tensor_tensor(
                            xrs[c].rearrange("p (i b) -> p i b", i=I),
                            xts[c].rearrange("p (i b) -> p i b", i=I),
                            rden.unsqueeze(1).broadcast_to([MCH, I, B]),
                            MULT,
                        )

                if stage == 15 and k == 1:
                    continue
                # ---------- s^T accumulation ----------
                sT_sb = wp.tile([128, ND], F32, tag="sTsb", bufs=1)
                if k == 0:
                    # uniform c: rhs (xt) is n-independent -> pack 8 n's in
                    # the stationary: lhsT [m, (n8, d)] -> out [(j,d), b]
                    for g in range(N // 8):
                        sacc8_t = pp.tile([MCH, 2048], F32, tag="phi", name="sacc8")
                        sacc8 = sacc8_t
                        for c in range(CH):
                            for i in range(I):
                                nc.tensor.matmul(
                                    sacc8[0:128, 0:B],
                                    lhsT=w0s[c][:, i * (N * D) + g * 8 * D:
                                                i * (N * D) + (g + 1) * 8 * D],
                                    rhs=xts[c][:, i * B:(i + 1) * B],
                                    start=(c == 0 and i == 0),
                                    stop=(c == CH - 1 and i == I - 1),
                                )
                        nc.scalar.copy(sT_sb[:, g * B:(g + 1) * B], sacc8[0:128, 0:B])
                else:
                    sT16 = wp.tile([16, N * B], F32, tag="sT16", bufs=1)
                    for np_ in range(N // 2):
                        n0 = np_ * 2
                        sacc_t = pp.tile([MCH, 2048], F32, tag="phi", name="sacc")
                        for n2 in range(2):
                            n = n0 + n2
                            for c in range(CH):
                                zn = zp.tile([MCH, I * B], F16, tag="zn", name="zn")
                                nc.vector.tensor_tensor(
                                    zn.rearrange("p (i b) -> p i b", i=I),
                                    xrs[c].rearrange("p (i b) -> p i b", i=I),
                                    expT[c][:, n * B:(n + 1) * B]
                                        .unsqueeze(1).broadcast_to([MCH, I, B]),
                                    MULT,
                                )
                                for i in range(I):
                                    nc.tensor.matmul(
                                        sacc_t[0:D, n2 * B:(n2 + 1) * B],
                                        lhsT=w0s[c][:, i * (N * D) + n * D:
                                                    i * (N * D) + (n + 1) * D],
                                        rhs=zn[:, i * B:(i + 1) * B],
                                        start=(c == 0 and i == 0),
                                        stop=(c == CH - 1 and i == I - 1),
                                    )
                        nc.scalar.copy(sT16[0:16, n0 * B:(n0 + 2) * B],
                                       sacc_t[0:D, 0:2 * B])
                # ---------- AllReduce of s^T partials ----------
                # ssq ends up [(j, d), (q, b)] with n = q*8 + j for every k.
                ssq = wp.tile([128, ND], F32, tag="ssq", bufs=1)
                if k == 0:
                    s_in = dp.tile([128, ND], F32, tag="sin", bufs=2)
                    s_out = dp.tile([128, ND], F32, tag="sout", bufs=2)
                    nc.sync.dma_start(s_in[:], sT_sb[:])
                    if no_collective:
                        nc.sync.dma_start(s_out[:], s_in[:])
                    else:
                        nc.gpsimd.collective_compute(
                            "AllReduce", ADD,
                            replica_groups=[list(range(CORES))],
                            ins=[s_in.opt()],
                            outs=[s_out.opt()],
                        )
                    nc.sync.dma_start(ssq[:], s_out[:])
                else:
                    s_in1 = dp.tile([16, N * B], F32, tag="sin1", bufs=2)
                    s_out1 = dp.tile([16, N * B], F32, tag="sout1", bufs=2)
                    H = N * B // 2
                    nc.sync.dma_start(s_in1[:, 0:H], sT16[:, 0:H])
                    nc.sync.dma_start(s_in1[:, H:], sT16[:, H:])
                    if no_collective:
                        nc.sync.dma_start(s_out1[:, 0:H], s_in1[:, 0:H])
                        nc.sync.dma_start(s_out1[:, H:], s_in1[:, H:])
                    else:
                        nc.gpsimd.collective_compute(
                            "AllReduce", ADD,
                            replica_groups=[list(range(CORES))],
                            ins=[s_in1.opt()],
                            outs=[s_out1.opt()],
                        )
                    # scatter [d, (q, j, b)] -> [(j, d), (q, b)] during readback
                    nc.sync.dma_start(
                        ssq[:],
                        s_out1.rearrange("d (q j b) -> j d q b", q=4, j=8),
                    )

                # ---------- squash (layout [(j,d), (q,b)], n = q*8+j) ----------
                kscale = (1.0 / N) if k == 0 else 1.0
                sq = wp.tile([128, ND], F16, tag="sqf", bufs=2)
                nc.scalar.activation(sq[:], ssq[:], ACT.Square, scale=kscale)
                s2_t = pp.tile([MCH, 2048], F32, tag="phi", name="ps_sq")
                s2 = s2_t[:, 0:ND]
                nc.tensor.matmul(s2, lhsT=bones[:], rhs=sq[:], start=True, stop=True)
                qq = wp.tile([128, ND], F32, tag="sqz", bufs=2)
                nc.scalar.activation(qq[:], s2, ACT.Sqrt, bias=eps_t[:])
                rr = wp.tile([128, ND], F32, tag="sqz", bufs=2)
                nc.vector.scalar_tensor_tensor(rr[:], s2, 1.0, qq[:], ADD, MULT)
                ww = wp.tile([128, ND], F32, tag="sqz", bufs=2)
                nc.vector.reciprocal(ww[:], rr[:])
                sc = wp.tile([128, ND], F32, tag="sqz", bufs=2)
                nc.vector.tensor_tensor(sc[:], s2, ww[:], MULT)
                o = wp.tile([128, ND], F16 if k < last_full else F32,
                            tag="ot" if k < last_full else "ot32", bufs=2)
                with nc.allow_low_precision(reason="outputs tolerate f16"):
                    nc.vector.scalar_tensor_tensor(o[:], ssq[:], kscale, sc[:], MULT, MULT)

                if k == last_full:
                    nc.sync.dma_start(out_d, o[:])
                if k < ks[-1]:
                    if k == 0:
                        nc.vector.tensor_copy(osum[:], o[:])
                    else:
                        nc.vector.tensor_tensor(osum[:], osum[:], o[:], ADD)
                    # Rsum^T rebuild: osum [(j,d),(q,b)] f16 -> DRAM in
                    # [d, (n,b)] layout (scatter on store), then contiguous
                    # replicating loads into the rt pair tiles.
                    o_dram = dp.tile([D, N * B], F16, tag="osd", bufs=2)
                    nc.sync.dma_start(
                        o_dram.rearrange("d (q j b) -> j d q b", q=4, j=8),
                        osum[:])
                    # rt rows 32q'+16par+d hold RsumT[d, (n,b)]
                    for qq_ in range(4):
                        for par in range(2):
                            rt_dst = rtA if par == 0 else rtB
                            row0 = 32 * qq_ + 16 * par
                            eng = nc.sync if par == 0 else nc.scalar
                            eng.dma_start(rt_dst[row0:row0 + 16, :], o_dram[:, :])

    nc.compile()
    return nc


def _host_prep(inputs: np.ndarray, W: np.ndarray):
    """Build the per-core input maps (all layouts host-side)."""
    inputs = np.ascontiguousarray(inputs, dtype=np.float32)
    W = np.ascontiguousarray(W, dtype=np.float32)
    bones = np.kron(np.eye(8, dtype=np.float32),
                    np.ones((16, 16), dtype=np.float32))
    in_maps = []
    for core in range(CORES):
        xc = inputs[:, core * MC:(core + 1) * MC, :]      # [B, MC, I]
        Wc = W[:, core * MC:(core + 1) * MC, :, :]        # [N, MC, D, I]
        # xt[m, (c, i, b)]: per chunk, (i, b) layout
        xcr = xc.reshape(B, CH, MCH, I)
        xt = xcr.transpose(2, 1, 3, 0).reshape(MCH, CH * I * B)
        # wphi[32q+16par+d, (n, c, m)] = W[n, m, d, i], i = 2q+par
        Wr = Wc.reshape(N, CH, MCH, D, I)
        wphi = np.zeros((4, 2, D, N, CH, MCH), dtype=np.float32)
        for i in range(I):
            q, par = i // 2, i % 2
            wphi[q, par] = Wr[:, :, :, :, i].transpose(3, 0, 1, 2)
        wphi = wphi.reshape(128, N * CH * MCH)
        # xt2[m, (c, q, n2, par, b)]: x replicated over the n-pair dim
        arr = xcr.transpose(2, 1, 3, 0).reshape(MCH, CH, 4, 2, B)
        xt2 = np.broadcast_to(arr[:, :, :, None, :, :],
                              (MCH, CH, 4, 2, 2, B)).reshape(MCH, CH * 2048)
        # w0[m, (c, i, n, d)]
        w0 = Wr.transpose(2, 1, 4, 0, 3).reshape(MCH, CH * I * N * D)
        in_maps.append({
            "xt": np.ascontiguousarray(xt, dtype=np.float16),
            "xt2": np.ascontiguousarray(xt2, dtype=np.float16),
            "wphi": np.ascontiguousarray(wphi, dtype=np.float16),
            "w0": np.ascontiguousarray(w0, dtype=np.float16),
            "bones": bones.astype(np.float16),
        })
    return in_maps


def _decode_out(out_f: np.ndarray) -> np.ndarray:
    # out_f [128, 512] in [(j, d), (q, b)] layout, n = q*8+j -> [b, n, d]
    arr = out_f.astype(np.float32).reshape(8, D, 4, B)    # j, d, q, b
    return np.ascontiguousarray(
        arr.transpose(3, 2, 0, 1).reshape(B, N, D))


def run(inputs: np.ndarray, W: np.ndarray, trace: bool = False):
    key = "nc"
    if key not in _CACHE:
        _CACHE[key] = _build_nc(False)
    nc = _CACHE[key]
    in_maps = _host_prep(inputs, W)
    res = run_bass_kernel_spmd(nc, in_maps, core_ids=list(range(CORES)), trace=trace)
    out = _decode_out(res.results[0]["out_f"])
    return out, res


def kernel(inputs: np.ndarray, W: np.ndarray) -> np.ndarray:
    out, _ = run(inputs, W, trace=False)
    return out


# revision 42
# speedup vs baseline: 1.0206x; 1.0012x over previous
# CapsuleLayer dynamic-routing kernel for 8x Trainium2 NeuronCores — v2.
#
# Problem: u_hat[b,n,m,d] = sum_i W[n,m,d,i] * x[b,m,i]; 3 routing iterations
#   c = softmax_n(blog); s[b,n,d] = sum_m c*u_hat; out = squash_d(s);
#   blog += sum_d out*u_hat
# with B=128, M=2048, I=8, N=32, D=16.
#
# Sharding: M (input capsules) split across 8 cores, 256 per core; only the
# small s[b,n,d] partial sums cross cores (AllReduce) once per iteration.
#
# v2 layout: m on SBUF partitions (two chunks of 128), fp16 compute tensors.
# Per routing pass k>0:
#   phi_{n,i}[m,b] = sum_d W[n,m,d,i]*Rsum[b,n,d]      (PE, K=32 masked-pair)
#   tmp  = phi (PSUM->SBUF f16 copy on Act)
#   tmp2 = tmp * xT                                     (DVE, fp16 2x mode)
#   blogT_n[m,b] = sum_i tmp2                           (DVE tree-add, 2x)
#   expT = exp(blogT)  [Act];  Z = sum_n expT  [DVE tree];  xr = xT / Z
#   z_n[m,(i,b)] = expT_n * xr                          (DVE 2x)
#   sT[(j,d),b] += w0[m,(i,n,d)]^T @ z_n                (PE fp16, PSUM acc)
# s AllReduce in [(j,d),(q,b)] layout (n = q*8+j), squash in-place, Rsum^T
# rebuilt via a DRAM round trip into the zero-masked rtA/rtB pair tiles.

import numpy as np

import concourse.bacc as bacc
import concourse.mybir as mybir
import concourse.tile as tile
from concourse.bass_utils import run_bass_kernel_spmd

B = 128          # batch (== SBUF partitions)
MTOT = 2048      # input capsules
I = 8            # input capsule dim
N = 32           # output capsules
D = 16           # output capsule dim
CORES = 8
MC = MTOT // CORES   # 256 input capsules per core
CH = 2               # m chunks of 128 per core
MCH = MC // CH       # 128
ND = N * D           # 512
EPS = 1e-7
ROUTINGS = 3

F32 = mybir.dt.float32
F16 = mybir.dt.float16
ADD = mybir.AluOpType.add
MULT = mybir.AluOpType.mult
AX_X = mybir.AxisListType.X
ACT = mybir.ActivationFunctionType

_CACHE = {}


def _build_nc(debug_outputs: bool = False, no_collective: bool = False, stage: int = 3):
    nc = bacc.Bacc("TRN2", target_bir_lowering=False, debug=False,
                   num_devices=1 if no_collective else CORES)

    xt_d = nc.dram_tensor("xt", [MCH, CH * I * B], F16, kind="ExternalInput").ap()
    xt2_d = nc.dram_tensor("xt2", [MCH, CH * 2048], F16, kind="ExternalInput").ap()
    wphi_d = nc.dram_tensor("wphi", [128, N * CH * MCH], F16, kind="ExternalInput").ap()
    w0_d = nc.dram_tensor("w0", [MCH, CH * I * N * D], F16, kind="ExternalInput").ap()
    bones_d = nc.dram_tensor("bones", [128, 128], F16, kind="ExternalInput").ap()
    out_d = nc.dram_tensor("out_f", [128, ND], F32, kind="ExternalOutput").ap()

    with tile.TileContext(nc) as tc:
        with tc.tile_pool(name="const", bufs=1) as cp, \
             tc.tile_pool(name="work", bufs=1) as wp, \
             tc.tile_pool(name="tmp4", bufs=4) as tp, \
             tc.tile_pool(name="zp", bufs=3) as zp, \
             tc.tile_pool(name="phip", bufs=2, space="PSUM") as pp, \
             tc.tile_pool(name="dram", bufs=2, space="DRAM") as dp:

            # ---- persistent SBUF ----
            xts = [cp.tile([MCH, I * B], F16, tag=f"xt{c}", name=f"xt{c}")
                   for c in range(CH)]
            xt2s = [cp.tile([MCH, 2048], F16, tag=f"xt2{c}", name=f"xt2{c}")
                    for c in range(CH)]
            wphi = cp.tile([128, N * CH * MCH], F16, tag="wphi")
            w0s = [cp.tile([MCH, I * N * D], F16, tag=f"w0{c}", name=f"w0{c}")
                   for c in range(CH)]
            bones = cp.tile([128, 128], F16, tag="bones")
            eps_t = cp.tile([128, 1], F32, tag="eps")
            shf_t = cp.tile([128, 1], F32, tag="shf")
            rtA = wp.tile([128, N * B], F16, tag="rtA")
            rtB = wp.tile([128, N * B], F16, tag="rtB")
            osum = wp.tile([128, ND], F16, tag="osum")
            blogT = [wp.tile([MCH, N * B], F16, tag=f"blogT{c}", name=f"blogT{c}")
                     for c in range(CH)]
            expT = [wp.tile([MCH, N * B], F16, tag=f"expT{c}", name=f"expT{c}")
                    for c in range(CH)]
            xrs = [wp.tile([MCH, I * B], F16, tag=f"xr{c}", name=f"xr{c}")
                   for c in range(CH)]

            for c in range(CH):
                nc.sync.dma_start(xts[c][:], xt_d[:, c * (I * B):(c + 1) * (I * B)])
                nc.sync.dma_start(w0s[c][:], w0_d[:, c * (I * N * D):(c + 1) * (I * N * D)])
            nc.sync.dma_start(bones[:], bones_d)
            for c in range(CH):
                nc.sync.dma_start(xt2s[c][:], xt2_d[:, c * 2048:(c + 1) * 2048])
            nc.sync.dma_start(wphi[:], wphi_d)
            nc.gpsimd.memset(eps_t[:], EPS)
            nc.gpsimd.memset(shf_t[:], -4.0)
            nc.gpsimd.memset(rtA[:].bitcast(F32), 0.0)
            nc.gpsimd.memset(rtB[:].bitcast(F32), 0.0)

            if stage == 1:
                ks = [0]
            elif stage in (15, 2):
                ks = [0, 1]
            else:
                ks = [0, 1, 2]
            last_full = 0 if stage in (1, 15) else ks[-1]
            for k in ks:
                # ---------- logits -> expT (k > 0) ----------
                if k > 0:
                    for c in range(CH):
                        for np_ in range(N // 2):
                            n0 = np_ * 2
                            # phi pair: [m, (q, n2, par, b)]; bank q holds only
                            # tile_position q (both n's of the pair).
                            phi = pp.tile([MCH, 2048], F32, tag="phi", name="phi")
                            for n2 in range(2):
                                n = n0 + n2
                                for i in range(I):
                                    q, par = i // 2, i % 2
                                    rt_src = rtA if par == 0 else rtB
                                    off = q * 512 + n2 * 256 + par * B
                                    nc.tensor.matmul(
                                        phi[:, off:off + B],
                                        lhsT=wphi[32 * q:32 * q + 32,
                                                  (n * CH + c) * MCH:(n * CH + c + 1) * MCH],
                                        rhs=rt_src[32 * q:32 * q + 32, n * B:(n + 1) * B],
                                        start=True, stop=True,
                                        tile_position=(32 * q, 0),
                                    )
                            # contiguous PSUM f32 -> SBUF f16 evacuation (2 n's)
                            tmp = tp.tile([MCH, 2048], F16, tag="tmp4", name="tmp")
                            nc.scalar.copy(tmp[:], phi[:])
                            # tmp2 = tmp * x  (x replicated over n2 host-side)
                            tmp2 = tp.tile([MCH, 2048], F16, tag="tmp2", name="tmp2")
                            nc.vector.tensor_tensor(tmp2[:], tmp[:], xt2s[c][:], MULT)
                            # tree-reduce over i = (q, par): q-halves twice, then par
                            v = tmp2.rearrange("p (q r) -> p q r", q=4)
                            t1 = tp.tile([MCH, 1024], F16, tag="t1", name="t1")
                            nc.vector.tensor_tensor(
                                t1.rearrange("p (q r) -> p q r", q=2),
                                v[:, 0:2], v[:, 2:4], ADD)
                            v = t1.rearrange("p (q r) -> p q r", q=2)
                            t2 = tp.tile([MCH, 512], F16, tag="t2", name="t2")
                            nc.vector.tensor_tensor(
                                t2.unsqueeze(1), v[:, 0:1], v[:, 1:2], ADD)
                            v = t2.rearrange("p (n2 par b) -> p n2 par b", n2=2, par=2)
                            nc.vector.tensor_tensor(
                                blogT[c][:, n0 * B:(n0 + 2) * B]
                                    .rearrange("p (n2 b) -> p n2 b", n2=2).unsqueeze(2),
                                v[:, :, 0:1], v[:, :, 1:2], ADD)
                            if np_ % 4 == 3:
                                qx = np_ // 4
                                nc.scalar.activation(
                                    expT[c][:, qx * 8 * B:(qx + 1) * 8 * B],
                                    blogT[c][:, qx * 8 * B:(qx + 1) * 8 * B],
                                    ACT.Exp, bias=shf_t[:])
                                if qx == 1:
                                    dta = wp.tile([MCH, 8 * B], F16, tag="dta",
                                                  name="dta", bufs=2)
                                    nc.vector.tensor_tensor(
                                        dta[:], expT[c][:, 0:8 * B],
                                        expT[c][:, 8 * B:16 * B], ADD)
                                elif qx == 3:
                                    dtb = wp.tile([MCH, 8 * B], F16, tag="dtb",
                                                  name="dtb", bufs=2)
                                    nc.vector.tensor_tensor(
                                        dtb[:], expT[c][:, 16 * B:24 * B],
                                        expT[c][:, 24 * B:32 * B], ADD)
                        d2 = wp.tile([MCH, 8 * B], F16, tag="d2", name="d2", bufs=2)
                        nc.vector.tensor_tensor(d2[:], dta[:], dtb[:], ADD)
                        d3 = wp.tile([MCH, 4 * B], F16, tag="d3", name="d3", bufs=1)
                        nc.vector.tensor_tensor(
                            d3[:], d2[:, 0:4 * B], d2[:, 4 * B:8 * B], ADD)
                        d4 = wp.tile([MCH, 2 * B], F16, tag="d4", name="d4", bufs=1)
                        nc.vector.tensor_tensor(
                            d4[:], d3[:, 0:2 * B], d3[:, 2 * B:4 * B], ADD)
                        zden = wp.tile([MCH, B], F32, tag="zden", name="zden", bufs=2)
                        nc.vector.tensor_tensor(
                            zden[:], d4[:, 0:B], d4[:, B:2 * B], ADD)
                        rden = wp.tile([MCH, B], F16, tag="rden", name="rden", bufs=2)
                        with nc.allow_low_precision(reason="routing weights tolerate f16"):
                            nc.vector.reciprocal(rden[:], zden[:])
                        nc.vector.tensor_tensor(
                            xrs[c].rearrange("p (i b) -> p i b", i=I),
                            xts[c].rearrange("p (i b) -> p i b", i=I),
                            rden.unsqueeze(1).broadcast_to([MCH, I, B]),
                            MULT,
                        )

                if stage == 15 and k == 1:
                    continue
                # ---------- s^T accumulation ----------
                sT_sb = wp.tile([128, ND], F32, tag="sTsb", bufs=1)
                if k == 0:
                    # uniform c: rhs (xt) is n-independent -> pack 8 n's in
                    # the stationary: lhsT [m, (n8, d)] -> out [(j,d), b]
                    for g in range(N // 8):
                        sacc8_t = pp.tile([MCH, 2048], F32, tag="phi", name="sacc8")
                        sacc8 = sacc8_t
                        for c in range(CH):
                            for i in range(I):
                                nc.tensor.matmul(
                                    sacc8[0:128, 0:B],
                                    lhsT=w0s[c][:, i * (N * D) + g * 8 * D:
                                                i * (N * D) + (g + 1) * 8 * D],
                                    rhs=xts[c][:, i * B:(i + 1) * B],
                                    start=(c == 0 and i == 0),
                                    stop=(c == CH - 1 and i == I - 1),
                                )
                        nc.scalar.copy(sT_sb[:, g * B:(g + 1) * B], sacc8[0:128, 0:B])
                else:
                    sT16 = wp.tile([16, N * B], F32, tag="sT16", bufs=1)
                    for np_ in range(N // 2):
                        n0 = np_ * 2
                        sacc_t = pp.tile([MCH, 2048], F32, tag="phi", name="sacc")
                        for n2 in range(2):
                            n = n0 + n2
                            for c in range(CH):
                                zn = zp.tile([MCH, I * B], F16, tag="zn", name="zn")
                                nc.vector.tensor_tensor(
                                    zn.rearrange("p (i b) -> p i b", i=I),
                                    xrs[c].rearrange("p (i b) -> p i b", i=I),
                                    expT[c][:, n * B:(n + 1) * B]
                                        .unsqueeze(1).broadcast_to([MCH, I, B]),
                                    MULT,
                                )
                                for i in range(I):
                                    nc.tensor.matmul(
                                        sacc_t[0:D, n2 * B:(n2 + 1) * B],
                                        lhsT=w0s[c][:, i * (N * D) + n * D:
                                                    i * (N * D) + (n + 1) * D],
                                        rhs=zn[:, i * B:(i + 1) * B],
                                        start=(c == 0 and i == 0),
                                        stop=(c == CH - 1 and i == I - 1),
                                    )
                        nc.scalar.copy(sT16[0:16, n0 * B:(n0 + 2) * B],
                                       sacc_t[0:D, 0:2 * B])
                # ---------- AllReduce of s^T partials ----------
                # ssq ends up [(j, d), (q, b)] with n = q*8 + j for every k.
                ssq = wp.tile([128, ND], F32, tag="ssq", bufs=1)
                if k == 0:
                    s_in = dp.tile([128, ND], F32, tag="sin", bufs=2)
                    s_out = dp.tile([128, ND], F32, tag="sout", bufs=2)
                    nc.sync.dma_start(s_in[:, 0:ND // 2], sT_sb[:, 0:ND // 2])
                    nc.sync.dma_start(s_in[:, ND // 2:], sT_sb[:, ND // 2:])
                    if no_collective:
                        nc.sync.dma_start(s_out[:, 0:ND // 2], s_in[:, 0:ND // 2])
                        nc.sync.dma_start(s_out[:, ND // 2:], s_in[:, ND // 2:])
                    else:
                        nc.gpsimd.collective_compute(
                            "AllReduce", ADD,
                            replica_groups=[list(range(CORES))],
                            ins=[s_in.opt()],
                            outs=[s_out.opt()],
                        )
                    nc.sync.dma_start(ssq[:, 0:ND // 2], s_out[:, 0:ND // 2])
                    nc.sync.dma_start(ssq[:, ND // 2:], s_out[:, ND // 2:])
                else:
                    s_in1 = dp.tile([16, N * B], F32, tag="sin1", bufs=2)
                    s_out1 = dp.tile([16, N * B], F32, tag="sout1", bufs=2)
                    H = N * B // 2
                    nc.sync.dma_start(s_in1[:, 0:H], sT16[:, 0:H])
                    nc.sync.dma_start(s_in1[:, H:], sT16[:, H:])
                    if no_collective:
                        nc.sync.dma_start(s_out1[:, 0:H], s_in1[:, 0:H])
                        nc.sync.dma_start(s_out1[:, H:], s_in1[:, H:])
                    else:
                        nc.gpsimd.collective_compute(
                            "AllReduce", ADD,
                            replica_groups=[list(range(CORES))],
                            ins=[s_in1.opt()],
                            outs=[s_out1.opt()],
                        )
                    # scatter [d, (q, j, b)] -> [(j, d), (q, b)] during readback
                    nc.sync.dma_start(
                        ssq[:],
                        s_out1.rearrange("d (q j b) -> j d q b", q=4, j=8),
                    )

                # ---------- squash (layout [(j,d), (q,b)], n = q*8+j) ----------
                kscale = (1.0 / N) if k == 0 else 1.0
                sq = wp.tile([128, ND], F16, tag="sqf", bufs=2)
                nc.scalar.activation(sq[:], ssq[:], ACT.Square, scale=kscale)
                s2_t = pp.tile([MCH, 2048], F32, tag="phi", name="ps_sq")
                s2 = s2_t[:, 0:ND]
                nc.tensor.matmul(s2, lhsT=bones[:], rhs=sq[:], start=True, stop=True)
                qq = wp.tile([128, ND], F32, tag="sqz", bufs=2)
                nc.scalar.activation(qq[:], s2, ACT.Sqrt, bias=eps_t[:])
                rr = wp.tile([128, ND], F32, tag="sqz", bufs=2)
                nc.vector.scalar_tensor_tensor(rr[:], s2, 1.0, qq[:], ADD, MULT)
                ww = wp.tile([128, ND], F32, tag="sqz", bufs=2)
                nc.vector.reciprocal(ww[:], rr[:])
                sc = wp.tile([128, ND], F32, tag="sqz", bufs=2)
                nc.vector.tensor_tensor(sc[:], s2, ww[:], MULT)
                if k == 0 and k < last_full:
                    o = osum
                else:
                    o = wp.tile([128, ND], F16 if k < last_full else F32,
                                tag="ot" if k < last_full else "ot32", bufs=2)
                with nc.allow_low_precision(reason="outputs tolerate f16"):
                    nc.vector.scalar_tensor_tensor(o[:], ssq[:], kscale, sc[:], MULT, MULT)

                if k == last_full:
                    nc.sync.dma_start(out_d, o[:])
                if k < ks[-1]:
                    if k > 0:
                        nc.vector.tensor_tensor(osum[:], osum[:], o[:], ADD)
                    # Rsum^T rebuild: osum [(j,d),(q,b)] f16 -> DRAM in
                    # [d, (n,b)] layout (scatter on store), then contiguous
                    # replicating loads into the rt pair tiles.
                    o_dram = dp.tile([D, N * B], F16, tag="osd", bufs=2)
                    nc.sync.dma_start(
                        o_dram.rearrange("d (q j b) -> j d q b", q=4, j=8),
                        osum[:])
                    # rt rows 32q'+16par+d hold RsumT[d, (n,b)]
                    for qq_ in range(4):
                        for par in range(2):
                            rt_dst = rtA if par == 0 else rtB
                            row0 = 32 * qq_ + 16 * par
                            eng = nc.sync if par == 0 else nc.scalar
                            eng.dma_start(rt_dst[row0:row0 + 16, :], o_dram[:, :])

    nc.compile()
    return nc


def _host_prep(inputs: np.ndarray, W: np.ndarray):
    """Build the per-core input maps (all layouts host-side)."""
    inputs = np.ascontiguousarray(inputs, dtype=np.float32)
    W = np.ascontiguousarray(W, dtype=np.float32)
    bones = np.kron(np.eye(8, dtype=np.float32),
                    np.ones((16, 16), dtype=np.float32))
    in_maps = []
    for core in range(CORES):
        xc = inputs[:, core * MC:(core + 1) * MC, :]      # [B, MC, I]
        Wc = W[:, core * MC:(core + 1) * MC, :, :]        # [N, MC, D, I]
        # xt[m, (c, i, b)]: per chunk, (i, b) layout
        xcr = xc.reshape(B, CH, MCH, I)
        xt = xcr.transpose(2, 1, 3, 0).reshape(MCH, CH * I * B)
        # wphi[32q+16par+d, (n, c, m)] = W[n, m, d, i], i = 2q+par
        Wr = Wc.reshape(N, CH, MCH, D, I)
        wphi = np.zeros((4, 2, D, N, CH, MCH), dtype=np.float32)
        for i in range(I):
            q, par = i // 2, i % 2
            wphi[q, par] = Wr[:, :, :, :, i].transpose(3, 0, 1, 2)
        wphi = wphi.reshape(128, N * CH * MCH)
        # xt2[m, (c, q, n2, par, b)]: x replicated over the n-pair dim
        arr = xcr.transpose(2, 1, 3, 0).reshape(MCH, CH, 4, 2, B)
        xt2 = np.broadcast_to(arr[:, :, :, None, :, :],
                              (MCH, CH, 4, 2, 2, B)).reshape(MCH, CH * 2048)
        # w0[m, (c, i, n, d)]
        w0 = Wr.transpose(2, 1, 4, 0, 3).reshape(MCH, CH * I * N * D)
        in_maps.append({
            "xt": np.ascontiguousarray(xt, dtype=np.float16),
            "xt2": np.ascontiguousarray(xt2, dtype=np.float16),
            "wphi": np.ascontiguousarray(wphi, dtype=np.float16),
            "w0": np.ascontiguousarray(w0, dtype=np.float16),
            "bones": bones.astype(np.float16),
        })
    return in_maps


def _decode_out(out_f: np.ndarray) -> np.ndarray:
    # out_f [128, 512] in [(j, d), (q, b)] layout, n = q*8+j -> [b, n, d]
    arr = out_f.astype(np.float32).reshape(8, D, 4, B)    # j, d, q, b
    return np.ascontiguousarray(
        arr.transpose(3, 2, 0, 1).reshape(B, N, D))


def run(inputs: np.ndarray, W: np.ndarray, trace: bool = False):
    key = "nc"
    if key not in _CACHE:
        _CACHE[key] = _build_nc(False)
    nc = _CACHE[key]
    in_maps = _host_prep(inputs, W)
    res = run_bass_kernel_spmd(nc, in_maps, core_ids=list(range(CORES)), trace=trace)
    out = _decode_out(res.results[0]["out_f"])
    return out, res


def kernel(inputs: np.ndarray, W: np.ndarray) -> np.ndarray:
    out, _ = run(inputs, W, trace=False)
    return out


# revision 43
# speedup vs baseline: 1.0218x; 1.0012x over previous
# CapsuleLayer dynamic-routing kernel for 8x Trainium2 NeuronCores — v2.
#
# Problem: u_hat[b,n,m,d] = sum_i W[n,m,d,i] * x[b,m,i]; 3 routing iterations
#   c = softmax_n(blog); s[b,n,d] = sum_m c*u_hat; out = squash_d(s);
#   blog += sum_d out*u_hat
# with B=128, M=2048, I=8, N=32, D=16.
#
# Sharding: M (input capsules) split across 8 cores, 256 per core; only the
# small s[b,n,d] partial sums cross cores (AllReduce) once per iteration.
#
# v2 layout: m on SBUF partitions (two chunks of 128), fp16 compute tensors.
# Per routing pass k>0:
#   phi_{n,i}[m,b] = sum_d W[n,m,d,i]*Rsum[b,n,d]      (PE, K=32 masked-pair)
#   tmp  = phi (PSUM->SBUF f16 copy on Act)
#   tmp2 = tmp * xT                                     (DVE, fp16 2x mode)
#   blogT_n[m,b] = sum_i tmp2                           (DVE tree-add, 2x)
#   expT = exp(blogT)  [Act];  Z = sum_n expT  [DVE tree];  xr = xT / Z
#   z_n[m,(i,b)] = expT_n * xr                          (DVE 2x)
#   sT[(j,d),b] += w0[m,(i,n,d)]^T @ z_n                (PE fp16, PSUM acc)
# s AllReduce in [(j,d),(q,b)] layout (n = q*8+j), squash in-place, Rsum^T
# rebuilt via a DRAM round trip into the zero-masked rtA/rtB pair tiles.

import numpy as np

import concourse.bacc as bacc
import concourse.mybir as mybir
import concourse.tile as tile
from concourse.bass_utils import run_bass_kernel_spmd

B = 128          # batch (== SBUF partitions)
MTOT = 2048      # input capsules
I = 8            # input capsule dim
N = 32           # output capsules
D = 16           # output capsule dim
CORES = 8
MC = MTOT // CORES   # 256 input capsules per core
CH = 2               # m chunks of 128 per core
MCH = MC // CH       # 128
ND = N * D           # 512
EPS = 1e-7
ROUTINGS = 3

F32 = mybir.dt.float32
F16 = mybir.dt.float16
ADD = mybir.AluOpType.add
MULT = mybir.AluOpType.mult
AX_X = mybir.AxisListType.X
ACT = mybir.ActivationFunctionType

_CACHE = {}


def _build_nc(debug_outputs: bool = False, no_collective: bool = False, stage: int = 3):
    nc = bacc.Bacc("TRN2", target_bir_lowering=False, debug=False,
                   num_devices=1 if no_collective else CORES)

    xt_d = nc.dram_tensor("xt", [MCH, CH * I * B], F16, kind="ExternalInput").ap()
    xt2_d = nc.dram_tensor("xt2", [MCH, CH * 2048], F16, kind="ExternalInput").ap()
    wphi_d = nc.dram_tensor("wphi", [128, N * CH * MCH], F16, kind="ExternalInput").ap()
    w0_d = nc.dram_tensor("w0", [MCH, CH * I * N * D], F16, kind="ExternalInput").ap()
    bones_d = nc.dram_tensor("bones", [128, 128], F16, kind="ExternalInput").ap()
    out_d = nc.dram_tensor("out_f", [128, ND], F32, kind="ExternalOutput").ap()

    with tile.TileContext(nc) as tc:
        with tc.tile_pool(name="const", bufs=1) as cp, \
             tc.tile_pool(name="work", bufs=1) as wp, \
             tc.tile_pool(name="tmp4", bufs=5) as tp, \
             tc.tile_pool(name="zp", bufs=4) as zp, \
             tc.tile_pool(name="phip", bufs=2, space="PSUM") as pp, \
             tc.tile_pool(name="dram", bufs=2, space="DRAM") as dp:

            # ---- persistent SBUF ----
            xts = [cp.tile([MCH, I * B], F16, tag=f"xt{c}", name=f"xt{c}")
                   for c in range(CH)]
            xt2s = [cp.tile([MCH, 2048], F16, tag=f"xt2{c}", name=f"xt2{c}")
                    for c in range(CH)]
            wphi = cp.tile([128, N * CH * MCH], F16, tag="wphi")
            w0s = [cp.tile([MCH, I * N * D], F16, tag=f"w0{c}", name=f"w0{c}")
                   for c in range(CH)]
            bones = cp.tile([128, 128], F16, tag="bones")
            eps_t = cp.tile([128, 1], F32, tag="eps")
            shf_t = cp.tile([128, 1], F32, tag="shf")
            rtA = wp.tile([128, N * B], F16, tag="rtA")
            rtB = wp.tile([128, N * B], F16, tag="rtB")
            osum = wp.tile([128, ND], F16, tag="osum")
            blogT = [wp.tile([MCH, N * B], F16, tag=f"blogT{c}", name=f"blogT{c}")
                     for c in range(CH)]
            expT = [wp.tile([MCH, N * B], F16, tag=f"expT{c}", name=f"expT{c}")
                    for c in range(CH)]
            xrs = [wp.tile([MCH, I * B], F16, tag=f"xr{c}", name=f"xr{c}")
                   for c in range(CH)]

            for c in range(CH):
                nc.sync.dma_start(xts[c][:], xt_d[:, c * (I * B):(c + 1) * (I * B)])
                nc.sync.dma_start(w0s[c][:], w0_d[:, c * (I * N * D):(c + 1) * (I * N * D)])
            nc.sync.dma_start(bones[:], bones_d)
            for c in range(CH):
                nc.sync.dma_start(xt2s[c][:], xt2_d[:, c * 2048:(c + 1) * 2048])
            nc.sync.dma_start(wphi[:], wphi_d)
            nc.gpsimd.memset(eps_t[:], EPS)
            nc.gpsimd.memset(shf_t[:], -4.0)
            nc.gpsimd.memset(rtA[:].bitcast(F32), 0.0)
            nc.gpsimd.memset(rtB[:].bitcast(F32), 0.0)

            if stage == 1:
                ks = [0]
            elif stage in (15, 2):
                ks = [0, 1]
            else:
                ks = [0, 1, 2]
            last_full = 0 if stage in (1, 15) else ks[-1]
            for k in ks:
                # ---------- logits -> expT (k > 0) ----------
                if k > 0:
                    for c in range(CH):
                        for np_ in range(N // 2):
                            n0 = np_ * 2
                            # phi pair: [m, (q, n2, par, b)]; bank q holds only
                            # tile_position q (both n's of the pair).
                            phi = pp.tile([MCH, 2048], F32, tag="phi", name="phi")
                            for n2 in range(2):
                                n = n0 + n2
                                for i in range(I):
                                    q, par = i // 2, i % 2
                                    rt_src = rtA if par == 0 else rtB
                                    off = q * 512 + n2 * 256 + par * B
                                    nc.tensor.matmul(
                                        phi[:, off:off + B],
                                        lhsT=wphi[32 * q:32 * q + 32,
                                                  (n * CH + c) * MCH:(n * CH + c + 1) * MCH],
                                        rhs=rt_src[32 * q:32 * q + 32, n * B:(n + 1) * B],
                                        start=True, stop=True,
                                        tile_position=(32 * q, 0),
                                    )
                            # contiguous PSUM f32 -> SBUF f16 evacuation (2 n's)
                            tmp = tp.tile([MCH, 2048], F16, tag="tmp4", name="tmp")
                            nc.scalar.copy(tmp[:], phi[:])
                            # tmp2 = tmp * x  (x replicated over n2 host-side)
                            tmp2 = tp.tile([MCH, 2048], F16, tag="tmp2", name="tmp2")
                            nc.vector.tensor_tensor(tmp2[:], tmp[:], xt2s[c][:], MULT)
                            # tree-reduce over i = (q, par): q-halves twice, then par
                            v = tmp2.rearrange("p (q r) -> p q r", q=4)
                            t1 = tp.tile([MCH, 1024], F16, tag="t1", name="t1")
                            nc.vector.tensor_tensor(
                                t1.rearrange("p (q r) -> p q r", q=2),
                                v[:, 0:2], v[:, 2:4], ADD)
                            v = t1.rearrange("p (q r) -> p q r", q=2)
                            t2 = tp.tile([MCH, 512], F16, tag="t2", name="t2")
                            nc.vector.tensor_tensor(
                                t2.unsqueeze(1), v[:, 0:1], v[:, 1:2], ADD)
                            v = t2.rearrange("p (n2 par b) -> p n2 par b", n2=2, par=2)
                            nc.vector.tensor_tensor(
                                blogT[c][:, n0 * B:(n0 + 2) * B]
                                    .rearrange("p (n2 b) -> p n2 b", n2=2).unsqueeze(2),
                                v[:, :, 0:1], v[:, :, 1:2], ADD)
                            if np_ % 4 == 3:
                                qx = np_ // 4
                                nc.scalar.activation(
                                    expT[c][:, qx * 8 * B:(qx + 1) * 8 * B],
                                    blogT[c][:, qx * 8 * B:(qx + 1) * 8 * B],
                                    ACT.Exp, bias=shf_t[:])
                                if qx == 1:
                                    dta = wp.tile([MCH, 8 * B], F16, tag="dta",
                                                  name="dta", bufs=2)
                                    nc.vector.tensor_tensor(
                                        dta[:], expT[c][:, 0:8 * B],
                                        expT[c][:, 8 * B:16 * B], ADD)
                                elif qx == 3:
                                    dtb = wp.tile([MCH, 8 * B], F16, tag="dtb",
                                                  name="dtb", bufs=2)
                                    nc.vector.tensor_tensor(
                                        dtb[:], expT[c][:, 16 * B:24 * B],
                                        expT[c][:, 24 * B:32 * B], ADD)
                        d2 = wp.tile([MCH, 8 * B], F16, tag="d2", name="d2", bufs=2)
                        nc.vector.tensor_tensor(d2[:], dta[:], dtb[:], ADD)
                        d3 = wp.tile([MCH, 4 * B], F16, tag="d3", name="d3", bufs=1)
                        nc.vector.tensor_tensor(
                            d3[:], d2[:, 0:4 * B], d2[:, 4 * B:8 * B], ADD)
                        d4 = wp.tile([MCH, 2 * B], F16, tag="d4", name="d4", bufs=1)
                        nc.vector.tensor_tensor(
                            d4[:], d3[:, 0:2 * B], d3[:, 2 * B:4 * B], ADD)
                        zden = wp.tile([MCH, B], F32, tag="zden", name="zden", bufs=2)
                        nc.vector.tensor_tensor(
                            zden[:], d4[:, 0:B], d4[:, B:2 * B], ADD)
                        rden = wp.tile([MCH, B], F16, tag="rden", name="rden", bufs=2)
                        with nc.allow_low_precision(reason="routing weights tolerate f16"):
                            nc.vector.reciprocal(rden[:], zden[:])
                        nc.vector.tensor_tensor(
                            xrs[c].rearrange("p (i b) -> p i b", i=I),
                            xts[c].rearrange("p (i b) -> p i b", i=I),
                            rden.unsqueeze(1).broadcast_to([MCH, I, B]),
                            MULT,
                        )

                if stage == 15 and k == 1:
                    continue
                # ---------- s^T accumulation ----------
                sT_sb = wp.tile([128, ND], F32, tag="sTsb", bufs=1)
                if k == 0:
                    # uniform c: rhs (xt) is n-independent -> pack 8 n's in
                    # the stationary: lhsT [m, (n8, d)] -> out [(j,d), b]
                    for g in range(N // 8):
                        sacc8_t = pp.tile([MCH, 2048], F32, tag="phi", name="sacc8")
                        sacc8 = sacc8_t
                        for c in range(CH):
                            for i in range(I):
                                nc.tensor.matmul(
                                    sacc8[0:128, 0:B],
                                    lhsT=w0s[c][:, i * (N * D) + g * 8 * D:
                                                i * (N * D) + (g + 1) * 8 * D],
                                    rhs=xts[c][:, i * B:(i + 1) * B],
                                    start=(c == 0 and i == 0),
                                    stop=(c == CH - 1 and i == I - 1),
                                )
                        nc.scalar.copy(sT_sb[:, g * B:(g + 1) * B], sacc8[0:128, 0:B])
                else:
                    sT16 = wp.tile([16, N * B], F32, tag="sT16", bufs=1)
                    for np_ in range(N // 2):
                        n0 = np_ * 2
                        sacc_t = pp.tile([MCH, 2048], F32, tag="phi", name="sacc")
                        for n2 in range(2):
                            n = n0 + n2
                            for c in range(CH):
                                zn = zp.tile([MCH, I * B], F16, tag="zn", name="zn")
                                nc.vector.tensor_tensor(
                                    zn.rearrange("p (i b) -> p i b", i=I),
                                    xrs[c].rearrange("p (i b) -> p i b", i=I),
                                    expT[c][:, n * B:(n + 1) * B]
                                        .unsqueeze(1).broadcast_to([MCH, I, B]),
                                    MULT,
                                )
                                for i in range(I):
                                    nc.tensor.matmul(
                                        sacc_t[0:D, n2 * B:(n2 + 1) * B],
                                        lhsT=w0s[c][:, i * (N * D) + n * D:
                                                    i * (N * D) + (n + 1) * D],
                                        rhs=zn[:, i * B:(i + 1) * B],
                                        start=(c == 0 and i == 0),
                                        stop=(c == CH - 1 and i == I - 1),
                                    )
                        nc.scalar.copy(sT16[0:16, n0 * B:(n0 + 2) * B],
                                       sacc_t[0:D, 0:2 * B])
                # ---------- AllReduce of s^T partials ----------
                # ssq ends up [(j, d), (q, b)] with n = q*8 + j for every k.
                ssq = wp.tile([128, ND], F32, tag="ssq", bufs=1)
                if k == 0:
                    s_in = dp.tile([128, ND], F32, tag="sin", bufs=2)
                    s_out = dp.tile([128, ND], F32, tag="sout", bufs=2)
                    nc.sync.dma_start(s_in[:, 0:ND // 2], sT_sb[:, 0:ND // 2])
                    nc.sync.dma_start(s_in[:, ND // 2:], sT_sb[:, ND // 2:])
                    if no_collective:
                        nc.sync.dma_start(s_out[:, 0:ND // 2], s_in[:, 0:ND // 2])
                        nc.sync.dma_start(s_out[:, ND // 2:], s_in[:, ND // 2:])
                    else:
                        nc.gpsimd.collective_compute(
                            "AllReduce", ADD,
                            replica_groups=[list(range(CORES))],
                            ins=[s_in.opt()],
                            outs=[s_out.opt()],
                        )
                    nc.sync.dma_start(ssq[:, 0:ND // 2], s_out[:, 0:ND // 2])
                    nc.sync.dma_start(ssq[:, ND // 2:], s_out[:, ND // 2:])
                else:
                    s_in1 = dp.tile([16, N * B], F32, tag="sin1", bufs=2)
                    s_out1 = dp.tile([16, N * B], F32, tag="sout1", bufs=2)
                    H = N * B // 2
                    nc.sync.dma_start(s_in1[:, 0:H], sT16[:, 0:H])
                    nc.sync.dma_start(s_in1[:, H:], sT16[:, H:])
                    if no_collective:
                        nc.sync.dma_start(s_out1[:, 0:H], s_in1[:, 0:H])
                        nc.sync.dma_start(s_out1[:, H:], s_in1[:, H:])
                    else:
                        nc.gpsimd.collective_compute(
                            "AllReduce", ADD,
                            replica_groups=[list(range(CORES))],
                            ins=[s_in1.opt()],
                            outs=[s_out1.opt()],
                        )
                    # scatter [d, (q, j, b)] -> [(j, d), (q, b)] during readback
                    nc.sync.dma_start(
                        ssq[:],
                        s_out1.rearrange("d (q j b) -> j d q b", q=4, j=8),
                    )

                # ---------- squash (layout [(j,d), (q,b)], n = q*8+j) ----------
                kscale = (1.0 / N) if k == 0 else 1.0
                sq = wp.tile([128, ND], F16, tag="sqf", bufs=2)
                nc.scalar.activation(sq[:], ssq[:], ACT.Square, scale=kscale)
                s2_t = pp.tile([MCH, 2048], F32, tag="phi", name="ps_sq")
                s2 = s2_t[:, 0:ND]
                nc.tensor.matmul(s2, lhsT=bones[:], rhs=sq[:], start=True, stop=True)
                qq = wp.tile([128, ND], F32, tag="sqz", bufs=2)
                nc.scalar.activation(qq[:], s2, ACT.Sqrt, bias=eps_t[:])
                rr = wp.tile([128, ND], F32, tag="sqz", bufs=2)
                nc.vector.scalar_tensor_tensor(rr[:], s2, 1.0, qq[:], ADD, MULT)
                ww = wp.tile([128, ND], F32, tag="sqz", bufs=2)
                nc.vector.reciprocal(ww[:], rr[:])
                sc = wp.tile([128, ND], F32, tag="sqz", bufs=2)
                nc.vector.tensor_tensor(sc[:], s2, ww[:], MULT)
                if k == 0 and k < last_full:
                    o = osum
                else:
                    o = wp.tile([128, ND], F16 if k < last_full else F32,
                                tag="ot" if k < last_full else "ot32", bufs=2)
                with nc.allow_low_precision(reason="outputs tolerate f16"):
                    nc.vector.scalar_tensor_tensor(o[:], ssq[:], kscale, sc[:], MULT, MULT)

                if k == last_full:
                    nc.sync.dma_start(out_d, o[:])
                if k < ks[-1]:
                    if k > 0:
                        nc.vector.tensor_tensor(osum[:], osum[:], o[:], ADD)
                    # Rsum^T rebuild: osum [(j,d),(q,b)] f16 -> DRAM in
                    # [d, (n,b)] layout (scatter on store), then contiguous
                    # replicating loads into the rt pair tiles.
                    o_dram = dp.tile([D, N * B], F16, tag="osd", bufs=2)
                    nc.sync.dma_start(
                        o_dram.rearrange("d (q j b) -> j d q b", q=4, j=8),
                        osum[:])
                    # rt rows 32q'+16par+d hold RsumT[d, (n,b)]
                    for qq_ in range(4):
                        for par in range(2):
                            rt_dst = rtA if par == 0 else rtB
                            row0 = 32 * qq_ + 16 * par
                            eng = nc.sync if par == 0 else nc.scalar
                            eng.dma_start(rt_dst[row0:row0 + 16, :], o_dram[:, :])

    nc.compile()
    return nc


def _host_prep(inputs: np.ndarray, W: np.ndarray):
    """Build the per-core input maps (all layouts host-side)."""
    inputs = np.ascontiguousarray(inputs, dtype=np.float32)
    W = np.ascontiguousarray(W, dtype=np.float32)
    bones = np.kron(np.eye(8, dtype=np.float32),
                    np.ones((16, 16), dtype=np.float32))
    in_maps = []
    for core in range(CORES):
        xc = inputs[:, core * MC:(core + 1) * MC, :]      # [B, MC, I]
        Wc = W[:, core * MC:(core + 1) * MC, :, :]        # [N, MC, D, I]
        # xt[m, (c, i, b)]: per chunk, (i, b) layout
        xcr = xc.reshape(B, CH, MCH, I)
        xt = xcr.transpose(2, 1, 3, 0).reshape(MCH, CH * I * B)
        # wphi[32q+16par+d, (n, c, m)] = W[n, m, d, i], i = 2q+par
        Wr = Wc.reshape(N, CH, MCH, D, I)
        wphi = np.zeros((4, 2, D, N, CH, MCH), dtype=np.float32)
        for i in range(I):
            q, par = i // 2, i % 2
            wphi[q, par] = Wr[:, :, :, :, i].transpose(3, 0, 1, 2)
        wphi = wphi.reshape(128, N * CH * MCH)
        # xt2[m, (c, q, n2, par, b)]: x replicated over the n-pair dim
        arr = xcr.transpose(2, 1, 3, 0).reshape(MCH, CH, 4, 2, B)
        xt2 = np.broadcast_to(arr[:, :, :, None, :, :],
                              (MCH, CH, 4, 2, 2, B)).reshape(MCH, CH * 2048)
        # w0[m, (c, i, n, d)]
        w0 = Wr.transpose(2, 1, 4, 0, 3).reshape(MCH, CH * I * N * D)
        in_maps.append({
            "xt": np.ascontiguousarray(xt, dtype=np.float16),
            "xt2": np.ascontiguousarray(xt2, dtype=np.float16),
            "wphi": np.ascontiguousarray(wphi, dtype=np.float16),
            "w0": np.ascontiguousarray(w0, dtype=np.float16),
            "bones": bones.astype(np.float16),
        })
    return in_maps


def _decode_out(out_f: np.ndarray) -> np.ndarray:
    # out_f [128, 512] in [(j, d), (q, b)] layout, n = q*8+j -> [b, n, d]
    arr = out_f.astype(np.float32).reshape(8, D, 4, B)    # j, d, q, b
    return np.ascontiguousarray(
        arr.transpose(3, 2, 0, 1).reshape(B, N, D))


def run(inputs: np.ndarray, W: np.ndarray, trace: bool = False):
    key = "nc"
    if key not in _CACHE:
        _CACHE[key] = _build_nc(False)
    nc = _CACHE[key]
    in_maps = _host_prep(inputs, W)
    res = run_bass_kernel_spmd(nc, in_maps, core_ids=list(range(CORES)), trace=trace)
    out = _decode_out(res.results[0]["out_f"])
    return out, res


def kernel(inputs: np.ndarray, W: np.ndarray) -> np.ndarray:
    out, _ = run(inputs, W, trace=False)
    return out
